# revision 10
# baseline (speedup 1.0000x reference)
"""Trainium2 Bass kernel for nn_Attention_77025943487081.

Sharding: batch (4) data-parallel x 2-way head tensor-parallel over 8 cores.
Core c handles batch c//2 and heads [8*(c%2), 8*(c%2)+8). Each core emits 4
partial c_proj outputs (one per head-pair, bf16); the host sums the 8 partials
per batch and adds the c_proj bias.

Numerics (validated in numpy against the f32 reference, rel err ~8e-3 vs the
2e-2 gate):
  - The folded per-head+cross-head projection matrices are I + C with C at
    0.02 scale.  q/k/v = bf16(x_slice) + (x8 + dx8) @ C8 / 32 where x8/dx8
    are fp8e4m3 value+residual and C8 = fp8(32*C) (the 32x pre-scale keeps
    C's entries out of fp8's subnormal range). The correction matmuls run in
    fp8 DoubleRow mode (256-deep contraction, 0.5 cycles/row).
  - Scores/PV/c_proj operands are bf16, accumulation always f32 PSUM.
  - The 1/sqrt(D) score scale is applied inside the softmax exp activation
    (out = exp(in * 0.125)), so q is staged unscaled.

Attention per head: scores^T [k, q] per 128-wide k-chunk -> exp on Act ->
PV in [d, q] orientation (po65[65, S] accumulator; row 64 collects softmax
denominators via a ones row appended to V). Wide PV matmuls (<=512 q
columns) keep the PE instruction count low — per-instruction semaphore and
issue overhead (~50-100 ns) is what kills many-small-matmul schedules. The
diagonal (causally masked) block's PV is deferred one k-chunk so the Pool
affine_select round-trip hides behind the next chunk's scores.

Schedule: the Act engine paces each head (~19 us of softmax exp vs ~16 us
of PE work), so the remaining phase-1 chains and the previous pair's
partial c_proj run as filler inside later heads' k-chunk loops. Each head's
normalization (1/den broadcast via a ones-column matmul) is deferred into
the next head's first k-chunk. The final pair's c_proj evacuates PSUM on
both DVE and Act (Act is idle by then).
"""

import numpy as np
import ml_dtypes
from contextlib import ExitStack

import concourse.bass as bass
import concourse.tile as tile
from concourse import bacc, mybir
from concourse.bass_utils import run_bass_kernel_spmd

F32 = mybir.dt.float32
F32R = mybir.dt.float32r
BF16 = mybir.dt.bfloat16
FP8 = mybir.dt.float8e4
DR = mybir.MatmulPerfMode.DoubleRow
ACT_EXP = mybir.ActivationFunctionType.Exp
ACT_COPY = mybir.ActivationFunctionType.Copy
MULT = mybir.AluOpType.mult
ADD = mybir.AluOpType.add

B, S, E, H, D, P = 4, 2048, 1024, 16, 64, 64
G = 8            # heads per core
NCORE = 8
NT = S // 128    # 16 sequence tiles
NPAIR = 5        # fp8 DoubleRow contraction pairs: 1280 = 5*256 rows (1089 used)
CSCALE = 32.0    # fp8 pre-scale on the correction matrices
INV_CS = 1.0 / CSCALE


def build_nc():
    nc = bacc.Bacc("TRN2", target_bir_lowering=False, debug=False, num_devices=NCORE)
    cqk8 = nc.dram_tensor("cqk8", [NPAIR, 128, 2, 1024], FP8, kind="ExternalInput").ap()
    cv8 = nc.dram_tensor("cv8", [NPAIR, 128, 2, 512], FP8, kind="ExternalInput").ap()
    xt8 = nc.dram_tensor("xt8", [NPAIR, 128, 2, S], FP8, kind="ExternalInput").ap()
    dxt8 = nc.dram_tensor("dxt8", [4, 128, 2, S], FP8, kind="ExternalInput").ap()
    xtid = nc.dram_tensor("xtid", [4, 128, S], BF16, kind="ExternalInput").ap()
    xnat = nc.dram_tensor("xnat", [NT, 128, 512], BF16, kind="ExternalInput").ap()
    wc16 = nc.dram_tensor("wc16", [4, 128, E], BF16, kind="ExternalInput").ap()
    onesb = nc.dram_tensor("onesb", [128, 128], BF16, kind="ExternalInput").ap()
    onesr = nc.dram_tensor("onesr", [1, 64], F32R, kind="ExternalInput").ap()
    zerob = nc.dram_tensor("zerob", [1, 512], BF16, kind="ExternalInput").ap()
    outp = nc.dram_tensor("outp", [4, S, E], BF16, kind="ExternalOutput").ap()

    with nc.allow_low_precision("bf16/fp8 staged operands; f32 PSUM accumulation"), \
         tile.TileContext(nc) as tc, ExitStack() as top:
        const_p = top.enter_context(tc.tile_pool(name="const", bufs=1))
        qk_p = top.enter_context(tc.tile_pool(name="qkt", bufs=1))
        vaug_p = top.enter_context(tc.tile_pool(name="vaug", bufs=1))
        oT_p = top.enter_context(tc.tile_pool(name="oT", bufs=1))

        cqk_sb = const_p.tile([128, NPAIR, 2, 1024], FP8)
        cv_sb = const_p.tile([128, NPAIR, 2, 512], FP8)
        xt_sb = const_p.tile([128, NPAIR, 2, S], FP8)
        dxt_sb = const_p.tile([128, 4, 2, S], FP8)
        xtid_sb = const_p.tile([128, 4, S], BF16)
        xnat_sb = const_p.tile([128, NT, 512], BF16)
        wc_sb = const_p.tile([128, 4, E], BF16)
        zrow = const_p.tile([1, 512], BF16)
        ones64 = const_p.tile([1, 64], F32R)

        qkt = [qk_p.tile([128, S], BF16, name=f"qkt{m}") for m in range(8)]
        v65 = vaug_p.tile([128, NT, G, 66], BF16)   # col 64 = ones (denominator)
        oT = oT_p.tile([128, 4, S], BF16)

        # --- DMAs, startup-critical first -------------------------------
        # stage 1: exactly what the first chains (m0/m4, all pairs) and the
        # first score chunks need
        for pr in range(NPAIR):
            nc.sync.dma_start(out=cqk_sb[:, pr, :, 0:128], in_=cqk8[pr][:, :, 0:128])
            nc.sync.dma_start(out=cqk_sb[:, pr, :, 512:640],
                              in_=cqk8[pr][:, :, 512:640])
            nc.sync.dma_start(out=xt_sb[:, pr, :, 0:1024],
                              in_=xt8[pr][:, :, 0:1024])
            if pr < 4:
                nc.sync.dma_start(out=dxt_sb[:, pr, :, 0:1024],
                                  in_=dxt8[pr][:, :, 0:1024])
        nc.sync.dma_start(out=xtid_sb[:, 0, 0:1024], in_=xtid[0][:, 0:1024])
        # stage 2: second halves + v-path + the rest
        for pr in range(NPAIR):
            nc.sync.dma_start(out=xt_sb[:, pr, :, 1024:2048],
                              in_=xt8[pr][:, :, 1024:2048])
            if pr < 4:
                nc.sync.dma_start(out=dxt_sb[:, pr, :, 1024:2048],
                                  in_=dxt8[pr][:, :, 1024:2048])
        nc.sync.dma_start(out=xtid_sb[:, 0, 1024:2048], in_=xtid[0][:, 1024:2048])
        for pr in range(NPAIR):
            nc.sync.dma_start(out=cv_sb[:, pr], in_=cv8[pr])
        for stt in range(4):
            nc.sync.dma_start(out=xnat_sb[:, stt], in_=xnat[stt])
        for pr in range(NPAIR):
            nc.sync.dma_start(out=cqk_sb[:, pr, :, 128:512],
                              in_=cqk8[pr][:, :, 128:512])
            nc.sync.dma_start(out=cqk_sb[:, pr, :, 640:1024],
                              in_=cqk8[pr][:, :, 640:1024])
        for m4 in range(1, 4):
            nc.sync.dma_start(out=xtid_sb[:, m4], in_=xtid[m4])
        for stt in range(4, NT):
            nc.sync.dma_start(out=xnat_sb[:, stt], in_=xnat[stt])
        for gc in range(4):
            nc.sync.dma_start(out=wc_sb[:, gc], in_=wc16[gc])
        nc.sync.dma_start(out=zrow, in_=zerob)
        nc.sync.dma_start(out=ones64, in_=onesr)
        for col in (64, 65):
            nc.sync.dma_start(
                out=v65[:, :, :, col:col + 1],
                in_=onesb[:, 0:128].rearrange("p (a b c) -> p a b c", a=16, b=8, c=1))

        with tc.tile_pool(name="stp", bufs=2, space="PSUM") as stp_p, \
             tc.tile_pool(name="pop", bufs=1, space="PSUM") as po_p, \
             tc.tile_pool(name="pt", bufs=2) as pt_p, \
             tc.tile_pool(name="diag", bufs=2) as diag_p, \
             tc.tile_pool(name="bcst", bufs=2) as bcst_p, \
             tc.tile_pool(name="rcpp", bufs=2) as rcp_p, \
             tc.tile_pool(name="ost", bufs=3) as ost_p:

            # ---------- phase-1 building blocks ----------
            def qk_chain(m, ss):
                """qkt[m][:, ss*512:+512] = identity x-slice + fp8 correction."""
                ps = stp_p.tile([128, 1024], F32, tag="stp", name=f"qkps{m}_{ss}")
                side = 0 if m < 4 else 512
                col0 = side + (m % 4) * 128
                xsl = slice(ss * 512, (ss + 1) * 512)
                for pr in range(NPAIR):
                    nc.tensor.matmul(ps[:, 0:512],
                                     cqk_sb[:, pr, :, col0:col0 + 128],
                                     xt_sb[:, pr, :, xsl],
                                     start=(pr == 0), stop=False, perf_mode=DR)
                for pr in range(4):
                    nc.tensor.matmul(ps[:, 0:512],
                                     cqk_sb[:, pr, :, col0:col0 + 128],
                                     dxt_sb[:, pr, :, xsl],
                                     start=False, stop=(pr == 3), perf_mode=DR)
                nc.vector.scalar_tensor_tensor(
                    out=qkt[m][:, xsl], in0=ps[:, 0:512], scalar=INV_CS,
                    in1=xtid_sb[:, m % 4, xsl], op0=MULT, op1=ADD)

            def v_chain(stt):
                pv = stp_p.tile([128, 1024], F32, tag="stp", name=f"vps{stt}")
                for pr in range(NPAIR):
                    nc.tensor.matmul(pv[:, 0:512],
                                     xt_sb[:, pr, :, stt * 128:(stt + 1) * 128],
                                     cv_sb[:, pr, :, :],
                                     start=(pr == 0), stop=(pr == NPAIR - 1),
                                     perf_mode=DR)
                nc.vector.scalar_tensor_tensor(
                    out=v65[:, stt, :, 0:64],
                    in0=pv[:, 0:512].rearrange("p (g d) -> p g d", g=G),
                    scalar=INV_CS,
                    in1=xnat_sb[:, stt].rearrange("p (g d) -> p g d", g=G),
                    op0=MULT, op1=ADD)

            def cproj_chunk(gc, stt, on_act=False):
                """one 128-row slab of the partial c_proj for head-pair gc"""
                pc = stp_p.tile([128, 1024], F32, tag="stp", name=f"pc{gc}_{stt}")
                for ee in range(2):
                    nc.tensor.matmul(pc[:, ee * 512:(ee + 1) * 512],
                                     oT[:, gc, stt * 128:(stt + 1) * 128],
                                     wc_sb[:, gc, ee * 512:(ee + 1) * 512],
                                     start=True, stop=True)
                ost = ost_p.tile([128, E], BF16, tag="ost", name=f"ost{gc}_{stt}")
                if on_act:
                    nc.scalar.activation(ost[:, :], pc[:, :], ACT_COPY)
                else:
                    nc.vector.tensor_copy(ost[:, :], pc[:, :])
                nc.sync.dma_start(out=outp[gc, stt * 128:(stt + 1) * 128, :],
                                  in_=ost[:, :])

            # ---------- phase-2 per-head attention ----------
            state = {"norm": None, "diag": None}

            def normalize(h, po):
                """oT[d-half, pair, :] = po65[0:64, :] / po65[64, :] (bf16)"""
                m, half = h // 2, h % 2
                rcp = rcp_p.tile([1, S], F32R, tag="rcp", name=f"rcp{h}")
                nc.vector.reciprocal(rcp, po[64:65, :])
                bcst = bcst_p.tile([64, S], BF16, tag="bcst", name=f"bcst{h}")
                for grp in range(4):
                    gs = slice(grp * 512, (grp + 1) * 512)
                    bc_ps = stp_p.tile([128, 1024], F32, tag="stp",
                                       name=f"bc{h}_{grp}")
                    nc.tensor.matmul(bc_ps[0:64, 0:512], ones64, rcp[:, gs],
                                     start=True, stop=True)
                    nc.vector.tensor_copy(bcst[:, gs], bc_ps[0:64, 0:512])
                nc.vector.tensor_mul(oT[64 * half:64 * half + 64, m, :],
                                     po[0:64, :], bcst[:, :])

            def head_attention(h, fillers):
                """fillers: dict kc -> list of thunks emitted at that k-chunk."""
                m, half = h // 2, h % 2
                qt = qkt[m][64 * half:64 * half + 64, :]
                kt = qkt[4 + m][64 * half:64 * half + 64, :]
                po = None
                for kc in range(NT):
                    q0 = 128 * kc
                    ptile = pt_p.tile([128, 2048], BF16, tag="pt",
                                      name=f"pt{h}_{kc}")
                    for c0 in range(q0, S, 1024):
                        cw = min(1024, S - c0)
                        st = stp_p.tile([128, 1024], F32, tag="stp",
                                        name=f"st{h}_{kc}_{c0}")
                        for u0 in range(c0, c0 + cw, 512):
                            uw = min(512, c0 + cw - u0)
                            nc.tensor.matmul(st[:, u0 - c0:u0 - c0 + uw],
                                             kt[:, q0:q0 + 128], qt[:, u0:u0 + uw],
                                             start=True, stop=True)
                        nc.scalar.activation(ptile[:, c0 - q0:c0 - q0 + cw],
                                             st[:, 0:cw], ACT_EXP, scale=0.125)
                    if kc == 0:
                        # previous head's deferred normalization must be
                        # emitted before its po slot is reallocated below
                        if state["norm"] is not None:
                            state["norm"]()
                            state["norm"] = None
                        po = po_p.tile([65, S], F32, tag="po", name=f"po{h}")
                        # zero bank 0 so every PV there can accumulate with
                        # start=False (PSUM pending-zero rules)
                        nc.tensor.matmul(po[:, 0:512], zrow[0:1, 0:65],
                                         zrow[0:1, 0:512],
                                         start=True, stop=False,
                                         skip_group_check=True)
                    dg = diag_p.tile([128, 128], BF16, tag="dg",
                                     name=f"dg{h}_{kc}")
                    nc.gpsimd.affine_select(
                        out=dg, in_=ptile[:, 0:128],
                        compare_op=mybir.AluOpType.is_ge, fill=0.0,
                        base=0, pattern=[[1, 128]], channel_multiplier=-1)
                    if state["diag"] is not None:
                        state["diag"]()
                        state["diag"] = None
                    # non-diagonal PV: q in [q0+128, S), 512-aligned pieces;
                    # kc==0 pieces outside bank 0 open their accumulation group
                    a = q0 + 128
                    while a < S:
                        b_ = min(S, (a // 512 + 1) * 512)
                        nc.tensor.matmul(po[:, a:b_], v65[:, kc, h, 0:65],
                                         ptile[:, a - q0:b_ - q0],
                                         start=(kc == 0 and a >= 512), stop=False,
                                         skip_group_check=True)
                        a = b_
                    state["diag"] = (
                        lambda kcc=kc, dgg=dg, poo=po, hh=h: nc.tensor.matmul(
                            poo[:, kcc * 128:(kcc + 1) * 128],
                            v65[:, kcc, hh, 0:65], dgg[:, :],
                            start=False, stop=True, skip_group_check=True))
                    for th in fillers.get(kc, ()):
                        th()
                state["diag"]()
                state["diag"] = None
                state["norm"] = lambda poo=po, hh=h: normalize(hh, poo)

            # ---------- schedule ----------
            # prefix: head 0's q must be complete (scores read qt[:, q0:S]);
            # its k and v arrive narrowly (kt[:, q0:q0+128] / v65[:, kc]) so
            # later chunks stream in as filler
            for m_, ss_ in ((0, 0), (0, 1), (4, 0), (0, 2), (0, 3)):
                qk_chain(m_, ss_)
            v_chain(0)

            def QK(m, ss):
                return lambda: qk_chain(m, ss)

            def VC(*stts):
                return lambda: [v_chain(s) for s in stts]

            def CP(gc, *stts):
                return lambda: [cproj_chunk(gc, s) for s in stts]

            # filler placement rules: a chain emitted at slot kc is only
            # readable from chunk kc+1 on (reads emitted before writes see
            # stale data).  v65[:, k] is needed at PV(k); kt chunk ss at
            # scores(4*ss); q chunks must be complete before the head starts.
            fillers = {
                0: {0: [VC(1)], 1: [QK(4, 1), VC(2)], 2: [VC(3, 4)],
                    3: [VC(5, 6)], 5: [QK(4, 2), VC(7, 8)], 7: [VC(9, 10)],
                    9: [QK(4, 3), VC(11, 12)], 11: [VC(13, 14)], 13: [VC(15)]},
                1: {1: [QK(1, 0)], 3: [QK(5, 0)], 5: [QK(1, 1)], 7: [QK(5, 1)],
                    9: [QK(1, 2)], 11: [QK(5, 2)], 13: [QK(1, 3), QK(5, 3)]},
                2: {1: [QK(2, 0)], 3: [QK(6, 0)], 5: [QK(2, 1)], 7: [QK(6, 1)],
                    9: [CP(0, 0, 1)], 11: [CP(0, 2, 3)], 13: [CP(0, 4, 5)]},
                3: {1: [QK(2, 2)], 3: [QK(6, 2)], 5: [QK(2, 3)], 7: [QK(6, 3)],
                    9: [CP(0, 6, 7)], 11: [CP(0, 8, 9, 10)],
                    13: [CP(0, 11, 12, 13)], 14: [CP(0, 14, 15)]},
                4: {1: [QK(3, 0)], 3: [QK(7, 0)], 5: [QK(3, 1)], 7: [QK(7, 1)],
                    9: [CP(1, 0, 1)], 11: [CP(1, 2, 3)], 13: [CP(1, 4, 5)]},
                5: {1: [QK(3, 2)], 3: [QK(7, 2)], 5: [QK(3, 3)], 7: [QK(7, 3)],
                    9: [CP(1, 6, 7)], 11: [CP(1, 8, 9, 10)],
                    13: [CP(1, 11, 12, 13)], 14: [CP(1, 14, 15)]},
                6: {1: [CP(2, 0)], 3: [CP(2, 1, 2)], 5: [CP(2, 3, 4)],
                    7: [CP(2, 5, 6)], 9: [CP(2, 7, 8)], 11: [CP(2, 9, 10)],
                    13: [CP(2, 11, 12)]},
                7: {1: [CP(2, 13)], 3: [CP(2, 14)], 5: [CP(2, 15)]},
            }
            for h in range(8):
                head_attention(h, fillers[h])
            state["norm"]()
            # tail: last pair's c_proj, PSUM evacuation split between DVE
            # and the now-idle Act engine
            for stt in range(NT):
                cproj_chunk(3, stt, on_act=(stt % 2 == 1))

    nc.compile()
    return nc


def _fp8(a, scale=1.0):
    return (np.asarray(a, np.float32) * scale).astype(ml_dtypes.float8_e4m3)


def _bf16(a):
    return np.asarray(a, np.float32).astype(ml_dtypes.bfloat16)


def prep_core_inputs(hidden_states, position_states, Wq, bq, Wqh, bqh, Wk, bk,
                     Wkh, bkh, Wv, bv, Wvh, bvh, Wp, bp, Wpe, bpe, Wc, bc):
    """Host-side weight folding + per-core staging."""
    f32 = np.float32
    eyeE = np.eye(E, dtype=f32)

    def fold(parity):
        hs = slice(G * parity, G * parity + G)
        csl = slice(512 * parity, 512 * parity + 512)
        mats = {}
        for name, (Wa, ba, Wh, bh, v) in {
            "q": (Wq, bq, Wqh[hs], bqh[hs], 0),
            "k": (Wk, bk, Wkh[hs], bkh[hs], 1),
            "v": (Wv, bv, Wvh[hs], bvh[hs], 2),
        }.items():
            mx = np.einsum("hed,ghd->hegd", Wa, Wh).reshape(E, 512)
            mp = np.einsum("pd,g->pgd", Wp[:, v * D:(v + 1) * D],
                           Wpe[v, 0, hs]).reshape(P, 512)
            bias = (np.einsum("hd,ghd->gd", ba, Wh) + bh
                    + bp[v * D:(v + 1) * D][None, :] * Wpe[v, 0, hs][:, None]
                    + bpe[hs][:, None]).reshape(512)
            C = np.zeros((NPAIR * 256, 512), f32)
            C[:E] = mx - eyeE[:, csl]
            C[E:E + P] = mp
            C[E + P] = bias
            mats[name] = C
        cqk = np.concatenate([mats["q"], mats["k"]], axis=1)     # [1280, 1024]
        cqk8 = _fp8(cqk.reshape(NPAIR, 2, 128, 1024).transpose(0, 2, 1, 3), CSCALE)
        cv8 = _fp8(mats["v"].reshape(NPAIR, 2, 128, 512).transpose(0, 2, 1, 3), CSCALE)
        wc = Wc.reshape(H, D, E)[hs].reshape(512, E).reshape(4, 128, E)
        return (np.ascontiguousarray(cqk8), np.ascontiguousarray(cv8),
                np.ascontiguousarray(_bf16(wc)))

    per_parity = [fold(0), fold(1)]

    in_maps = []
    for c in range(NCORE):
        b, parity = c // 2, c % 2
        csl = slice(512 * parity, 512 * parity + 512)
        xaug = np.zeros((NPAIR * 256, S), f32)
        xaug[:E] = hidden_states[b].T
        xaug[E:E + P] = position_states[b].T
        xaug[E + P] = 1.0
        x8 = _fp8(xaug)
        dx = xaug[:E] - x8[:E].astype(f32)
        xt8 = np.ascontiguousarray(
            x8.reshape(NPAIR, 2, 128, S).transpose(0, 2, 1, 3))
        dxt8 = np.ascontiguousarray(
            _fp8(dx).reshape(4, 2, 128, S).transpose(0, 2, 1, 3))
        xtid = np.ascontiguousarray(
            _bf16(hidden_states[b].T[csl]).reshape(4, 128, S))
        xnat = np.ascontiguousarray(
            _bf16(hidden_states[b][:, csl]).reshape(NT, 128, 512))
        cqk8, cv8, wc = per_parity[parity]
        in_maps.append({"cqk8": cqk8, "cv8": cv8, "xt8": xt8, "dxt8": dxt8,
                        "xtid": xtid, "xnat": xnat, "wc16": wc,
                        "onesb": _bf16(np.ones((128, 128), f32)),
                        "onesr": np.ones((1, 64), f32),
                        "zerob": _bf16(np.zeros((1, 512), f32))})
    return in_maps


_NC_CACHE = {}


def get_nc():
    if "nc" not in _NC_CACHE:
        _NC_CACHE["nc"] = build_nc()
    return _NC_CACHE["nc"]


def assemble(results, bc):
    outs = []
    for b in range(B):
        acc = np.zeros((S, E), np.float32)
        for p in range(2):
            acc += results[2 * b + p]["outp"].astype(np.float32).sum(axis=0)
        outs.append(acc + bc)
    return np.stack(outs).astype(np.float32)


def kernel(**inputs):
    nc = get_nc()
    in_maps = prep_core_inputs(**inputs)
    res = run_bass_kernel_spmd(nc, in_maps, list(range(NCORE)))
    return assemble(res.results, inputs["bc"])


# revision 11
# speedup vs baseline: 1.0549x; 1.0549x over previous
"""Trainium2 Bass kernel for nn_Attention_77025943487081.

Sharding: batch (4) data-parallel x 2-way head tensor-parallel over 8 cores.
Core c handles batch c//2 and heads [8*(c%2), 8*(c%2)+8). Each core emits 4
partial c_proj outputs (one per head-pair, bf16); the host sums the 8 partials
per batch and adds the c_proj bias.

Numerics (validated in numpy against the f32 reference, rel err ~8e-3 vs the
2e-2 gate):
  - The folded per-head+cross-head projection matrices are I + C with C at
    0.02 scale.  q/k/v = bf16(x_slice) + (x8 + dx8) @ C8 / 32 where x8/dx8
    are fp8e4m3 value+residual and C8 = fp8(32*C) (the 32x pre-scale keeps
    C's entries out of fp8's subnormal range). The correction matmuls run in
    fp8 DoubleRow mode (256-deep contraction, 0.5 cycles/row).
  - Scores/PV/c_proj operands are bf16, accumulation always f32 PSUM.
  - The 1/sqrt(D) score scale is applied inside the softmax exp activation
    (out = exp(in * 0.125)), so q is staged unscaled.

Attention per head: scores^T [k, q] per 128-wide k-chunk -> exp on Act ->
PV in [d, q] orientation (po65[65, S] accumulator; row 64 collects softmax
denominators via a ones row appended to V). Wide PV matmuls (<=512 q
columns) keep the PE instruction count low — per-instruction semaphore and
issue overhead (~50-100 ns) is what kills many-small-matmul schedules. The
diagonal (causally masked) block's PV is deferred one k-chunk so the Pool
affine_select round-trip hides behind the next chunk's scores.

Schedule: the Act engine paces each head (~19 us of softmax exp vs ~16 us
of PE work), so the remaining phase-1 chains and the previous pair's
partial c_proj run as filler inside later heads' k-chunk loops. Each head's
normalization (1/den broadcast via a ones-column matmul) is deferred into
the next head's first k-chunk. The final pair's c_proj evacuates PSUM on
both DVE and Act (Act is idle by then).
"""

import numpy as np
import ml_dtypes
from contextlib import ExitStack

import concourse.bass as bass
import concourse.tile as tile
from concourse import bacc, mybir
from concourse.bass_utils import run_bass_kernel_spmd

F32 = mybir.dt.float32
F32R = mybir.dt.float32r
BF16 = mybir.dt.bfloat16
FP8 = mybir.dt.float8e4
DR = mybir.MatmulPerfMode.DoubleRow
ACT_EXP = mybir.ActivationFunctionType.Exp
ACT_COPY = mybir.ActivationFunctionType.Copy
MULT = mybir.AluOpType.mult
ADD = mybir.AluOpType.add

B, S, E, H, D, P = 4, 2048, 1024, 16, 64, 64
G = 8            # heads per core
NCORE = 8
NT = S // 128    # 16 sequence tiles
NPAIR = 5        # fp8 DoubleRow contraction pairs: 1280 = 5*256 rows (1089 used)
CSCALE = 32.0    # fp8 pre-scale on the correction matrices
INV_CS = 1.0 / CSCALE


def build_nc():
    nc = bacc.Bacc("TRN2", target_bir_lowering=False, debug=False, num_devices=NCORE)
    cqk8 = nc.dram_tensor("cqk8", [NPAIR, 128, 2, 1024], FP8, kind="ExternalInput").ap()
    cv8 = nc.dram_tensor("cv8", [NPAIR, 128, 2, 512], FP8, kind="ExternalInput").ap()
    xt8 = nc.dram_tensor("xt8", [NPAIR, 128, 2, S], FP8, kind="ExternalInput").ap()
    dxt8 = nc.dram_tensor("dxt8", [4, 128, 2, S], FP8, kind="ExternalInput").ap()
    xtid = nc.dram_tensor("xtid", [4, 128, S], BF16, kind="ExternalInput").ap()
    xnat = nc.dram_tensor("xnat", [NT, 128, 512], BF16, kind="ExternalInput").ap()
    wc16 = nc.dram_tensor("wc16", [4, 128, E], BF16, kind="ExternalInput").ap()
    onesb = nc.dram_tensor("onesb", [128, 128], BF16, kind="ExternalInput").ap()
    onesr = nc.dram_tensor("onesr", [1, 64], F32R, kind="ExternalInput").ap()
    zerob = nc.dram_tensor("zerob", [1, 512], BF16, kind="ExternalInput").ap()
    outp = nc.dram_tensor("outp", [4, S, E], BF16, kind="ExternalOutput").ap()

    with nc.allow_low_precision("bf16/fp8 staged operands; f32 PSUM accumulation"), \
         tile.TileContext(nc) as tc, ExitStack() as top:
        const_p = top.enter_context(tc.tile_pool(name="const", bufs=1))
        qk_p = top.enter_context(tc.tile_pool(name="qkt", bufs=1))
        vaug_p = top.enter_context(tc.tile_pool(name="vaug", bufs=1))
        oT_p = top.enter_context(tc.tile_pool(name="oT", bufs=1))

        cqk_sb = const_p.tile([128, NPAIR, 2, 1024], FP8)
        cv_sb = const_p.tile([128, NPAIR, 2, 512], FP8)
        xt_sb = const_p.tile([128, NPAIR, 2, S], FP8)
        dxt_sb = const_p.tile([128, 4, 2, S], FP8)
        xtid_sb = const_p.tile([128, 4, S], BF16)
        xnat_sb = const_p.tile([128, NT, 512], BF16)
        wc_sb = const_p.tile([128, 4, E], BF16)
        zrow = const_p.tile([1, 512], BF16)
        ones64 = const_p.tile([1, 64], F32R)

        qkt = [qk_p.tile([128, S], BF16, name=f"qkt{m}") for m in range(8)]
        v65 = vaug_p.tile([128, NT, G, 66], BF16)   # col 64 = ones (denominator)
        oT = oT_p.tile([128, 4, S], BF16)

        # --- DMAs, startup-critical first -------------------------------
        nc.sync.dma_start(out=zrow, in_=zerob)
        # stage 1: exactly what the first chains (m0/m4, all pairs) and the
        # first score chunks need
        for pr in range(NPAIR):
            nc.sync.dma_start(out=cqk_sb[:, pr, :, 0:128], in_=cqk8[pr][:, :, 0:128])
            nc.sync.dma_start(out=cqk_sb[:, pr, :, 512:640],
                              in_=cqk8[pr][:, :, 512:640])
            nc.sync.dma_start(out=xt_sb[:, pr, :, 0:1024],
                              in_=xt8[pr][:, :, 0:1024])
            if pr < 4:
                nc.sync.dma_start(out=dxt_sb[:, pr, :, 0:1024],
                                  in_=dxt8[pr][:, :, 0:1024])
        nc.sync.dma_start(out=xtid_sb[:, 0, 0:1024], in_=xtid[0][:, 0:1024])
        # stage 2: second halves + v-path + the rest
        for pr in range(NPAIR):
            nc.sync.dma_start(out=xt_sb[:, pr, :, 1024:2048],
                              in_=xt8[pr][:, :, 1024:2048])
            if pr < 4:
                nc.sync.dma_start(out=dxt_sb[:, pr, :, 1024:2048],
                                  in_=dxt8[pr][:, :, 1024:2048])
        nc.sync.dma_start(out=xtid_sb[:, 0, 1024:2048], in_=xtid[0][:, 1024:2048])
        for pr in range(NPAIR):
            nc.sync.dma_start(out=cv_sb[:, pr], in_=cv8[pr])
        for stt in range(NT):
            nc.sync.dma_start(out=xnat_sb[:, stt], in_=xnat[stt])
        for pr in range(NPAIR):
            nc.sync.dma_start(out=cqk_sb[:, pr, :, 128:512],
                              in_=cqk8[pr][:, :, 128:512])
            nc.sync.dma_start(out=cqk_sb[:, pr, :, 640:1024],
                              in_=cqk8[pr][:, :, 640:1024])
        for m4 in range(1, 4):
            nc.sync.dma_start(out=xtid_sb[:, m4], in_=xtid[m4])
        for gc in range(4):
            nc.sync.dma_start(out=wc_sb[:, gc], in_=wc16[gc])
        nc.sync.dma_start(out=ones64, in_=onesr)
        for col in (64, 65):
            nc.sync.dma_start(
                out=v65[:, :, :, col:col + 1],
                in_=onesb[:, 0:128].rearrange("p (a b c) -> p a b c", a=16, b=8, c=1))

        with tc.tile_pool(name="stp", bufs=2, space="PSUM") as stp_p, \
             tc.tile_pool(name="pop", bufs=1, space="PSUM") as po_p, \
             tc.tile_pool(name="pt", bufs=2) as pt_p, \
             tc.tile_pool(name="diag", bufs=2) as diag_p, \
             tc.tile_pool(name="bcst", bufs=2) as bcst_p, \
             tc.tile_pool(name="rcpp", bufs=2) as rcp_p, \
             tc.tile_pool(name="ost", bufs=3) as ost_p:

            # ---------- phase-1 building blocks ----------
            def qk_chain(m, ss):
                """qkt[m][:, ss*512:+512] = identity x-slice + fp8 correction."""
                ps = stp_p.tile([128, 1024], F32, tag="stp", name=f"qkps{m}_{ss}")
                side = 0 if m < 4 else 512
                col0 = side + (m % 4) * 128
                xsl = slice(ss * 512, (ss + 1) * 512)
                for pr in range(NPAIR):
                    nc.tensor.matmul(ps[:, 0:512],
                                     cqk_sb[:, pr, :, col0:col0 + 128],
                                     xt_sb[:, pr, :, xsl],
                                     start=(pr == 0), stop=False, perf_mode=DR)
                for pr in range(4):
                    nc.tensor.matmul(ps[:, 0:512],
                                     cqk_sb[:, pr, :, col0:col0 + 128],
                                     dxt_sb[:, pr, :, xsl],
                                     start=False, stop=(pr == 3), perf_mode=DR)
                nc.vector.scalar_tensor_tensor(
                    out=qkt[m][:, xsl], in0=ps[:, 0:512], scalar=INV_CS,
                    in1=xtid_sb[:, m % 4, xsl], op0=MULT, op1=ADD)

            def v_chain(stt):
                pv = stp_p.tile([128, 1024], F32, tag="stp", name=f"vps{stt}")
                for pr in range(NPAIR):
                    nc.tensor.matmul(pv[:, 0:512],
                                     xt_sb[:, pr, :, stt * 128:(stt + 1) * 128],
                                     cv_sb[:, pr, :, :],
                                     start=(pr == 0), stop=(pr == NPAIR - 1),
                                     perf_mode=DR)
                nc.vector.scalar_tensor_tensor(
                    out=v65[:, stt, :, 0:64],
                    in0=pv[:, 0:512].rearrange("p (g d) -> p g d", g=G),
                    scalar=INV_CS,
                    in1=xnat_sb[:, stt].rearrange("p (g d) -> p g d", g=G),
                    op0=MULT, op1=ADD)

            def cproj_chunk(gc, stt, on_act=False):
                """one 128-row slab of the partial c_proj for head-pair gc"""
                pc = stp_p.tile([128, 1024], F32, tag="stp", name=f"pc{gc}_{stt}")
                for ee in range(2):
                    nc.tensor.matmul(pc[:, ee * 512:(ee + 1) * 512],
                                     oT[:, gc, stt * 128:(stt + 1) * 128],
                                     wc_sb[:, gc, ee * 512:(ee + 1) * 512],
                                     start=True, stop=True)
                ost = ost_p.tile([128, E], BF16, tag="ost", name=f"ost{gc}_{stt}")
                if on_act:
                    nc.scalar.activation(ost[:, :], pc[:, :], ACT_COPY)
                else:
                    nc.vector.tensor_copy(ost[:, :], pc[:, :])
                nc.sync.dma_start(out=outp[gc, stt * 128:(stt + 1) * 128, :],
                                  in_=ost[:, :])

            # ---------- phase-2 per-head attention ----------
            state = {"norm": None, "diag": None, "pv": None}

            def normalize(h, po):
                """oT[d-half, pair, :] = po65[0:64, :] / po65[64, :] (bf16)"""
                m, half = h // 2, h % 2
                rcp = rcp_p.tile([1, S], F32R, tag="rcp", name=f"rcp{h}")
                nc.vector.reciprocal(rcp, po[64:65, :])
                bcst = bcst_p.tile([64, S], BF16, tag="bcst", name=f"bcst{h}")
                for grp in range(4):
                    gs = slice(grp * 512, (grp + 1) * 512)
                    bc_ps = stp_p.tile([128, 1024], F32, tag="stp",
                                       name=f"bc{h}_{grp}")
                    nc.tensor.matmul(bc_ps[0:64, 0:512], ones64, rcp[:, gs],
                                     start=True, stop=True)
                    nc.vector.tensor_copy(bcst[:, gs], bc_ps[0:64, 0:512])
                nc.vector.tensor_mul(oT[64 * half:64 * half + 64, m, :],
                                     po[0:64, :], bcst[:, :])

            def head_attention(h, fillers):
                """fillers: dict kc -> list of thunks emitted at that k-chunk."""
                m, half = h // 2, h % 2
                qt = qkt[m][64 * half:64 * half + 64, :]
                kt = qkt[4 + m][64 * half:64 * half + 64, :]
                po = None
                for kc in range(NT):
                    q0 = 128 * kc
                    ptile = pt_p.tile([128, 2048], BF16, tag="pt",
                                      name=f"pt{h}_{kc}")
                    for c0 in range(q0, S, 1024):
                        cw = min(1024, S - c0)
                        st = stp_p.tile([128, 1024], F32, tag="stp",
                                        name=f"st{h}_{kc}_{c0}")
                        for u0 in range(c0, c0 + cw, 512):
                            uw = min(512, c0 + cw - u0)
                            nc.tensor.matmul(st[:, u0 - c0:u0 - c0 + uw],
                                             kt[:, q0:q0 + 128], qt[:, u0:u0 + uw],
                                             start=True, stop=True)
                        nc.scalar.activation(ptile[:, c0 - q0:c0 - q0 + cw],
                                             st[:, 0:cw], ACT_EXP, scale=0.125)
                    if kc == 0:
                        # previous head's deferred normalization must be
                        # emitted before its po slot is reallocated below
                        if state["norm"] is not None:
                            state["norm"]()
                            state["norm"] = None
                        po = po_p.tile([65, S], F32, tag="po", name=f"po{h}")
                        # zero bank 0 so every PV there can accumulate with
                        # start=False (PSUM pending-zero rules)
                        nc.tensor.matmul(po[:, 0:512], zrow[0:1, 0:65],
                                         zrow[0:1, 0:512],
                                         start=True, stop=False,
                                         skip_group_check=True)
                    dg = diag_p.tile([128, 128], BF16, tag="dg",
                                     name=f"dg{h}_{kc}")
                    nc.gpsimd.affine_select(
                        out=dg, in_=ptile[:, 0:128],
                        compare_op=mybir.AluOpType.is_ge, fill=0.0,
                        base=0, pattern=[[1, 128]], channel_multiplier=-1)
                    # PV runs one k-chunk behind the scores so the PE never
                    # waits on the exp it just requested: PV(kc-1) reads a
                    # ptile whose exp finished during scores(kc)
                    if state["pv"] is not None:
                        state["pv"]()
                        state["pv"] = None
                    if state["diag"] is not None:
                        state["diag"]()
                        state["diag"] = None

                    def pv_pieces(kcc=kc, ptt=ptile, poo=po, hh=h):
                        # non-diagonal PV: q in [q0+128, S), 512-aligned
                        # pieces; kc==0 pieces outside bank 0 open their
                        # accumulation group (bank 0 was pre-zeroed)
                        a = 128 * kcc + 128
                        while a < S:
                            b_ = min(S, (a // 512 + 1) * 512)
                            nc.tensor.matmul(poo[:, a:b_],
                                             v65[:, kcc, hh, 0:65],
                                             ptt[:, a - 128 * kcc:b_ - 128 * kcc],
                                             start=(kcc == 0 and a >= 512),
                                             stop=False, skip_group_check=True)
                            a = b_

                    state["pv"] = pv_pieces
                    state["diag"] = (
                        lambda kcc=kc, dgg=dg, poo=po, hh=h: nc.tensor.matmul(
                            poo[:, kcc * 128:(kcc + 1) * 128],
                            v65[:, kcc, hh, 0:65], dgg[:, :],
                            start=False, stop=True, skip_group_check=True))
                    for th in fillers.get(kc, ()):
                        th()
                state["pv"]()
                state["pv"] = None
                state["diag"]()
                state["diag"] = None
                state["norm"] = lambda poo=po, hh=h: normalize(hh, poo)

            # ---------- schedule ----------
            # warm the PE clock (p-state ramps over ~3us of busy time)
            # while the input DMAs land
            for w in range(8):
                wt = stp_p.tile([128, 1024], F32, tag="stp", name=f"warm{w}")
                nc.tensor.matmul(wt[:, 0:512], zrow[0:1, 0:128],
                                 zrow[0:1, 0:512], start=True, stop=True,
                                 skip_group_check=True)
            # prefix: head 0's q must be complete (scores read qt[:, q0:S]);
            # its k and v arrive narrowly (kt[:, q0:q0+128] / v65[:, kc]) so
            # later chunks stream in as filler
            for m_, ss_ in ((0, 0), (0, 1), (4, 0), (0, 2), (0, 3)):
                qk_chain(m_, ss_)
            v_chain(0)

            def QK(m, ss):
                return lambda: qk_chain(m, ss)

            def VC(*stts):
                return lambda: [v_chain(s) for s in stts]

            def CP(gc, *stts):
                return lambda: [cproj_chunk(gc, s) for s in stts]

            # filler placement rules: a chain emitted at slot kc is only
            # readable from chunk kc+1 on (reads emitted before writes see
            # stale data).  v65[:, k] is needed at PV(k); kt chunk ss at
            # scores(4*ss); q chunks must be complete before the head starts.
            fillers = {
                0: {0: [VC(1)], 1: [QK(4, 1), VC(2)], 2: [VC(3, 4)],
                    3: [VC(5, 6)], 5: [QK(4, 2), VC(7, 8)], 7: [VC(9, 10)],
                    9: [QK(4, 3), VC(11, 12)], 11: [VC(13, 14)], 13: [VC(15)]},
                1: {1: [QK(1, 0)], 3: [QK(5, 0)], 5: [QK(1, 1)], 7: [QK(5, 1)],
                    9: [QK(1, 2)], 11: [QK(5, 2)], 13: [QK(1, 3), QK(5, 3)]},
                2: {1: [QK(2, 0)], 3: [QK(6, 0)], 5: [QK(2, 1)], 7: [QK(6, 1)],
                    9: [CP(0, 0, 1)], 11: [CP(0, 2, 3)], 13: [CP(0, 4, 5)]},
                3: {1: [QK(2, 2)], 3: [QK(6, 2)], 5: [QK(2, 3)], 7: [QK(6, 3)],
                    9: [CP(0, 6, 7)], 11: [CP(0, 8, 9, 10)],
                    13: [CP(0, 11, 12, 13)], 14: [CP(0, 14, 15)]},
                4: {1: [QK(3, 0)], 3: [QK(7, 0)], 5: [QK(3, 1)], 7: [QK(7, 1)],
                    9: [CP(1, 0, 1)], 11: [CP(1, 2, 3)], 13: [CP(1, 4, 5)]},
                5: {1: [QK(3, 2)], 3: [QK(7, 2)], 5: [QK(3, 3)], 7: [QK(7, 3)],
                    9: [CP(1, 6, 7)], 11: [CP(1, 8, 9, 10)],
                    13: [CP(1, 11, 12, 13)], 14: [CP(1, 14, 15)]},
                6: {1: [CP(2, 0)], 3: [CP(2, 1, 2)], 5: [CP(2, 3, 4)],
                    7: [CP(2, 5, 6)], 9: [CP(2, 7, 8)], 11: [CP(2, 9, 10)],
                    13: [CP(2, 11, 12)]},
                7: {1: [CP(2, 13)], 3: [CP(2, 14)], 5: [CP(2, 15)]},
            }
            for h in range(8):
                head_attention(h, fillers[h])
            state["norm"]()
            # tail: last pair's c_proj, PSUM evacuation split between DVE
            # and the now-idle Act engine
            for stt in range(NT):
                cproj_chunk(3, stt, on_act=(stt % 2 == 1))

    nc.compile()
    return nc


def _fp8(a, scale=1.0):
    return (np.asarray(a, np.float32) * scale).astype(ml_dtypes.float8_e4m3)


def _bf16(a):
    return np.asarray(a, np.float32).astype(ml_dtypes.bfloat16)


def prep_core_inputs(hidden_states, position_states, Wq, bq, Wqh, bqh, Wk, bk,
                     Wkh, bkh, Wv, bv, Wvh, bvh, Wp, bp, Wpe, bpe, Wc, bc):
    """Host-side weight folding + per-core staging."""
    f32 = np.float32
    eyeE = np.eye(E, dtype=f32)

    def fold(parity):
        hs = slice(G * parity, G * parity + G)
        csl = slice(512 * parity, 512 * parity + 512)
        mats = {}
        for name, (Wa, ba, Wh, bh, v) in {
            "q": (Wq, bq, Wqh[hs], bqh[hs], 0),
            "k": (Wk, bk, Wkh[hs], bkh[hs], 1),
            "v": (Wv, bv, Wvh[hs], bvh[hs], 2),
        }.items():
            mx = np.einsum("hed,ghd->hegd", Wa, Wh).reshape(E, 512)
            mp = np.einsum("pd,g->pgd", Wp[:, v * D:(v + 1) * D],
                           Wpe[v, 0, hs]).reshape(P, 512)
            bias = (np.einsum("hd,ghd->gd", ba, Wh) + bh
                    + bp[v * D:(v + 1) * D][None, :] * Wpe[v, 0, hs][:, None]
                    + bpe[hs][:, None]).reshape(512)
            C = np.zeros((NPAIR * 256, 512), f32)
            C[:E] = mx - eyeE[:, csl]
            C[E:E + P] = mp
            C[E + P] = bias
            mats[name] = C
        cqk = np.concatenate([mats["q"], mats["k"]], axis=1)     # [1280, 1024]
        cqk8 = _fp8(cqk.reshape(NPAIR, 2, 128, 1024).transpose(0, 2, 1, 3), CSCALE)
        cv8 = _fp8(mats["v"].reshape(NPAIR, 2, 128, 512).transpose(0, 2, 1, 3), CSCALE)
        wc = Wc.reshape(H, D, E)[hs].reshape(512, E).reshape(4, 128, E)
        return (np.ascontiguousarray(cqk8), np.ascontiguousarray(cv8),
                np.ascontiguousarray(_bf16(wc)))

    per_parity = [fold(0), fold(1)]

    in_maps = []
    for c in range(NCORE):
        b, parity = c // 2, c % 2
        csl = slice(512 * parity, 512 * parity + 512)
        xaug = np.zeros((NPAIR * 256, S), f32)
        xaug[:E] = hidden_states[b].T
        xaug[E:E + P] = position_states[b].T
        xaug[E + P] = 1.0
        x8 = _fp8(xaug)
        dx = xaug[:E] - x8[:E].astype(f32)
        xt8 = np.ascontiguousarray(
            x8.reshape(NPAIR, 2, 128, S).transpose(0, 2, 1, 3))
        dxt8 = np.ascontiguousarray(
            _fp8(dx).reshape(4, 2, 128, S).transpose(0, 2, 1, 3))
        xtid = np.ascontiguousarray(
            _bf16(hidden_states[b].T[csl]).reshape(4, 128, S))
        xnat = np.ascontiguousarray(
            _bf16(hidden_states[b][:, csl]).reshape(NT, 128, 512))
        cqk8, cv8, wc = per_parity[parity]
        in_maps.append({"cqk8": cqk8, "cv8": cv8, "xt8": xt8, "dxt8": dxt8,
                        "xtid": xtid, "xnat": xnat, "wc16": wc,
                        "onesb": _bf16(np.ones((128, 128), f32)),
                        "onesr": np.ones((1, 64), f32),
                        "zerob": _bf16(np.zeros((1, 512), f32))})
    return in_maps


_NC_CACHE = {}


def get_nc():
    if "nc" not in _NC_CACHE:
        _NC_CACHE["nc"] = build_nc()
    return _NC_CACHE["nc"]


def assemble(results, bc):
    outs = []
    for b in range(B):
        acc = np.zeros((S, E), np.float32)
        for p in range(2):
            acc += results[2 * b + p]["outp"].astype(np.float32).sum(axis=0)
        outs.append(acc + bc)
    return np.stack(outs).astype(np.float32)


def kernel(**inputs):
    nc = get_nc()
    in_maps = prep_core_inputs(**inputs)
    res = run_bass_kernel_spmd(nc, in_maps, list(range(NCORE)))
    return assemble(res.results, inputs["bc"])


# revision 12
# speedup vs baseline: 1.1551x; 1.0950x over previous
"""Trainium2 Bass kernel for nn_Attention_77025943487081.

Sharding: batch (4) data-parallel x 2-way head tensor-parallel over 8 cores.
Core c handles batch c//2 and heads [8*(c%2), 8*(c%2)+8). Each core emits 4
partial c_proj outputs (one per head-pair, bf16); the host sums the 8 partials
per batch and adds the c_proj bias.

Numerics (validated in numpy against the f32 reference, rel err ~8e-3 vs the
2e-2 gate):
  - The folded per-head+cross-head projection matrices are I + C with C at
    0.02 scale.  q/k/v = bf16(x_slice) + (x8 + dx8) @ C8 / 32 where x8/dx8
    are fp8e4m3 value+residual and C8 = fp8(32*C) (the 32x pre-scale keeps
    C's entries out of fp8's subnormal range). The correction matmuls run in
    fp8 DoubleRow mode (256-deep contraction, 0.5 cycles/row).
  - Scores/PV/c_proj operands are bf16, accumulation always f32 PSUM.
  - The 1/sqrt(D) score scale is applied inside the softmax exp activation
    (out = exp(in * 0.125)), so q is staged unscaled.

Attention per head: scores^T [k, q] per 128-wide k-chunk -> exp on Act ->
PV in [d, q] orientation (po65[65, S] accumulator; row 64 collects softmax
denominators via a ones row appended to V). Wide PV matmuls (<=512 q
columns) keep the PE instruction count low — per-instruction semaphore and
issue overhead (~50-100 ns) is what kills many-small-matmul schedules. The
diagonal (causally masked) block's PV is deferred one k-chunk so the Pool
affine_select round-trip hides behind the next chunk's scores.

Schedule: the Act engine paces each head (~19 us of softmax exp vs ~16 us
of PE work), so the remaining phase-1 chains and the previous pair's
partial c_proj run as filler inside later heads' k-chunk loops. Each head's
normalization (1/den broadcast via a ones-column matmul) is deferred into
the next head's first k-chunk. The final pair's c_proj evacuates PSUM on
both DVE and Act (Act is idle by then).
"""

import numpy as np
import ml_dtypes
from contextlib import ExitStack

import concourse.bass as bass
import concourse.tile as tile
from concourse import bacc, mybir
from concourse.bass_utils import run_bass_kernel_spmd

F32 = mybir.dt.float32
F32R = mybir.dt.float32r
BF16 = mybir.dt.bfloat16
FP8 = mybir.dt.float8e4
DR = mybir.MatmulPerfMode.DoubleRow
ACT_EXP = mybir.ActivationFunctionType.Exp
ACT_COPY = mybir.ActivationFunctionType.Copy
MULT = mybir.AluOpType.mult
ADD = mybir.AluOpType.add

B, S, E, H, D, P = 4, 2048, 1024, 16, 64, 64
G = 8            # heads per core
NCORE = 8
NT = S // 128    # 16 sequence tiles
NPAIR = 5        # fp8 DoubleRow contraction pairs: 1280 = 5*256 rows (1089 used)
CSCALE = 32.0    # fp8 pre-scale on the correction matrices
INV_CS = 1.0 / CSCALE


def build_nc():
    nc = bacc.Bacc("TRN2", target_bir_lowering=False, debug=False, num_devices=NCORE)
    cqk8 = nc.dram_tensor("cqk8", [NPAIR, 128, 2, 1024], FP8, kind="ExternalInput").ap()
    cv8 = nc.dram_tensor("cv8", [NPAIR, 128, 2, 512], FP8, kind="ExternalInput").ap()
    xt8 = nc.dram_tensor("xt8", [NPAIR, 128, 2, S], FP8, kind="ExternalInput").ap()
    dxt8 = nc.dram_tensor("dxt8", [4, 128, 2, S], FP8, kind="ExternalInput").ap()
    xtid = nc.dram_tensor("xtid", [4, 128, S], BF16, kind="ExternalInput").ap()
    xnat = nc.dram_tensor("xnat", [NT, 128, 512], BF16, kind="ExternalInput").ap()
    wc16 = nc.dram_tensor("wc16", [4, 128, E], BF16, kind="ExternalInput").ap()
    onesb = nc.dram_tensor("onesb", [128, 128], BF16, kind="ExternalInput").ap()
    onesr = nc.dram_tensor("onesr", [1, 64], F32R, kind="ExternalInput").ap()
    zerob = nc.dram_tensor("zerob", [1, 512], BF16, kind="ExternalInput").ap()
    outp = nc.dram_tensor("outp", [2, S, E], BF16, kind="ExternalOutput").ap()

    with nc.allow_low_precision("bf16/fp8 staged operands; f32 PSUM accumulation"), \
         tile.TileContext(nc) as tc, ExitStack() as top:
        const_p = top.enter_context(tc.tile_pool(name="const", bufs=1))
        qk_p = top.enter_context(tc.tile_pool(name="qkt", bufs=1))
        vaug_p = top.enter_context(tc.tile_pool(name="vaug", bufs=1))
        oT_p = top.enter_context(tc.tile_pool(name="oT", bufs=1))

        cqk_sb = const_p.tile([128, NPAIR, 2, 1024], FP8)
        cv_sb = const_p.tile([128, NPAIR, 2, 512], FP8)
        xt_sb = const_p.tile([128, NPAIR, 2, S], FP8)
        dxt_sb = const_p.tile([128, 4, 2, S], FP8)
        xtid_sb = const_p.tile([128, 4, S], BF16)
        xnat_sb = const_p.tile([128, NT, 512], BF16)
        wc_sb = const_p.tile([128, 4, E], BF16)
        zrow = const_p.tile([1, 512], BF16)
        ones64 = const_p.tile([1, 64], F32R)

        qkt = [qk_p.tile([128, S], BF16, name=f"qkt{m}") for m in range(8)]
        v65 = vaug_p.tile([128, NT, G, 66], BF16)   # col 64 = ones (denominator)
        oT = oT_p.tile([128, 4, S], BF16)

        # --- DMAs, startup-critical first -------------------------------
        nc.sync.dma_start(out=zrow, in_=zerob)
        # stage 1: exactly what the first chains (m0/m4, all pairs) and the
        # first score chunks need
        for pr in range(NPAIR):
            nc.sync.dma_start(out=cqk_sb[:, pr, :, 0:128], in_=cqk8[pr][:, :, 0:128])
            nc.sync.dma_start(out=cqk_sb[:, pr, :, 512:640],
                              in_=cqk8[pr][:, :, 512:640])
            nc.sync.dma_start(out=xt_sb[:, pr, :, 0:1024],
                              in_=xt8[pr][:, :, 0:1024])
            if pr < 4:
                nc.sync.dma_start(out=dxt_sb[:, pr, :, 0:1024],
                                  in_=dxt8[pr][:, :, 0:1024])
        nc.sync.dma_start(out=xtid_sb[:, 0, 0:1024], in_=xtid[0][:, 0:1024])
        # stage 2: second halves + v-path + the rest
        for pr in range(NPAIR):
            nc.sync.dma_start(out=xt_sb[:, pr, :, 1024:2048],
                              in_=xt8[pr][:, :, 1024:2048])
            if pr < 4:
                nc.sync.dma_start(out=dxt_sb[:, pr, :, 1024:2048],
                                  in_=dxt8[pr][:, :, 1024:2048])
        nc.sync.dma_start(out=xtid_sb[:, 0, 1024:2048], in_=xtid[0][:, 1024:2048])
        for pr in range(NPAIR):
            nc.sync.dma_start(out=cv_sb[:, pr], in_=cv8[pr])
        for stt in range(NT):
            nc.sync.dma_start(out=xnat_sb[:, stt], in_=xnat[stt])
        for pr in range(NPAIR):
            nc.sync.dma_start(out=cqk_sb[:, pr, :, 128:512],
                              in_=cqk8[pr][:, :, 128:512])
            nc.sync.dma_start(out=cqk_sb[:, pr, :, 640:1024],
                              in_=cqk8[pr][:, :, 640:1024])
        for m4 in range(1, 4):
            nc.sync.dma_start(out=xtid_sb[:, m4], in_=xtid[m4])
        for gc in range(4):
            nc.sync.dma_start(out=wc_sb[:, gc], in_=wc16[gc])
        nc.sync.dma_start(out=ones64, in_=onesr)
        for col in (64, 65):
            nc.sync.dma_start(
                out=v65[:, :, :, col:col + 1],
                in_=onesb[:, 0:128].rearrange("p (a b c) -> p a b c", a=16, b=8, c=1))

        with tc.tile_pool(name="stp", bufs=2, space="PSUM") as stp_p, \
             tc.tile_pool(name="pop", bufs=1, space="PSUM") as po_p, \
             tc.tile_pool(name="pt", bufs=2) as pt_p, \
             tc.tile_pool(name="diag", bufs=2) as diag_p, \
             tc.tile_pool(name="bcst", bufs=2) as bcst_p, \
             tc.tile_pool(name="rcpp", bufs=2) as rcp_p, \
             tc.tile_pool(name="ost", bufs=3) as ost_p:

            # ---------- phase-1 building blocks ----------
            def qk_chain(m, ss):
                """qkt[m][:, ss*512:+512] = identity x-slice + fp8 correction."""
                ps = stp_p.tile([128, 1024], F32, tag="stp", name=f"qkps{m}_{ss}")
                side = 0 if m < 4 else 512
                col0 = side + (m % 4) * 128
                xsl = slice(ss * 512, (ss + 1) * 512)
                for pr in range(NPAIR):
                    nc.tensor.matmul(ps[:, 0:512],
                                     cqk_sb[:, pr, :, col0:col0 + 128],
                                     xt_sb[:, pr, :, xsl],
                                     start=(pr == 0), stop=False, perf_mode=DR)
                for pr in range(4):
                    nc.tensor.matmul(ps[:, 0:512],
                                     cqk_sb[:, pr, :, col0:col0 + 128],
                                     dxt_sb[:, pr, :, xsl],
                                     start=False, stop=(pr == 3), perf_mode=DR)
                nc.vector.scalar_tensor_tensor(
                    out=qkt[m][:, xsl], in0=ps[:, 0:512], scalar=INV_CS,
                    in1=xtid_sb[:, m % 4, xsl], op0=MULT, op1=ADD)

            def v_chain(stt):
                pv = stp_p.tile([128, 1024], F32, tag="stp", name=f"vps{stt}")
                for pr in range(NPAIR):
                    nc.tensor.matmul(pv[:, 0:512],
                                     xt_sb[:, pr, :, stt * 128:(stt + 1) * 128],
                                     cv_sb[:, pr, :, :],
                                     start=(pr == 0), stop=(pr == NPAIR - 1),
                                     perf_mode=DR)
                nc.vector.scalar_tensor_tensor(
                    out=v65[:, stt, :, 0:64],
                    in0=pv[:, 0:512].rearrange("p (g d) -> p g d", g=G),
                    scalar=INV_CS,
                    in1=xnat_sb[:, stt].rearrange("p (g d) -> p g d", g=G),
                    op0=MULT, op1=ADD)

            def cproj_chunk(part, stt, on_act=False):
                """one 128-row slab of the half c_proj (head-pairs 2p, 2p+1)"""
                pc = stp_p.tile([128, 1024], F32, tag="stp", name=f"pc{part}_{stt}")
                for gi, gc in enumerate((2 * part, 2 * part + 1)):
                    for ee in range(2):
                        nc.tensor.matmul(pc[:, ee * 512:(ee + 1) * 512],
                                         oT[:, gc, stt * 128:(stt + 1) * 128],
                                         wc_sb[:, gc, ee * 512:(ee + 1) * 512],
                                         start=(gi == 0), stop=(gi == 1),
                                         skip_group_check=True)
                ost = ost_p.tile([128, E], BF16, tag="ost", name=f"ost{part}_{stt}")
                if on_act:
                    nc.scalar.activation(ost[:, :], pc[:, :], ACT_COPY)
                else:
                    nc.vector.tensor_copy(ost[:, :], pc[:, :])
                nc.sync.dma_start(out=outp[part, stt * 128:(stt + 1) * 128, :],
                                  in_=ost[:, :])

            # ---------- phase-2 per-head attention ----------
            state = {"norm": None, "diag": None, "pv": None}

            def normalize(h, po):
                """oT[d-half, pair, :] = po65[0:64, :] / po65[64, :] (bf16)"""
                m, half = h // 2, h % 2
                rcp = rcp_p.tile([1, S], F32R, tag="rcp", name=f"rcp{h}")
                nc.vector.reciprocal(rcp, po[64:65, :])
                bcst = bcst_p.tile([64, S], BF16, tag="bcst", name=f"bcst{h}")
                for grp in range(4):
                    gs = slice(grp * 512, (grp + 1) * 512)
                    bc_ps = stp_p.tile([128, 1024], F32, tag="stp",
                                       name=f"bc{h}_{grp}")
                    nc.tensor.matmul(bc_ps[0:64, 0:512], ones64, rcp[:, gs],
                                     start=True, stop=True)
                    nc.vector.tensor_copy(bcst[:, gs], bc_ps[0:64, 0:512])
                nc.vector.tensor_mul(oT[64 * half:64 * half + 64, m, :],
                                     po[0:64, :], bcst[:, :])

            def head_attention(h, fillers):
                """fillers: dict kc -> list of thunks emitted at that k-chunk."""
                m, half = h // 2, h % 2
                qt = qkt[m][64 * half:64 * half + 64, :]
                kt = qkt[4 + m][64 * half:64 * half + 64, :]
                po = None
                for kc in range(NT):
                    q0 = 128 * kc
                    ptile = pt_p.tile([128, 2048], BF16, tag="pt",
                                      name=f"pt{h}_{kc}")
                    for c0 in range(q0, S, 1024):
                        cw = min(1024, S - c0)
                        st = stp_p.tile([128, 1024], F32, tag="stp",
                                        name=f"st{h}_{kc}_{c0}")
                        for u0 in range(c0, c0 + cw, 512):
                            uw = min(512, c0 + cw - u0)
                            nc.tensor.matmul(st[:, u0 - c0:u0 - c0 + uw],
                                             kt[:, q0:q0 + 128], qt[:, u0:u0 + uw],
                                             start=True, stop=True)
                        nc.scalar.activation(ptile[:, c0 - q0:c0 - q0 + cw],
                                             st[:, 0:cw], ACT_EXP, scale=0.125)
                    if kc == 0:
                        # previous head's deferred normalization must be
                        # emitted before its po slot is reallocated below
                        if state["norm"] is not None:
                            state["norm"]()
                            state["norm"] = None
                        po = po_p.tile([65, S], F32, tag="po", name=f"po{h}")
                        # zero bank 0 so every PV there can accumulate with
                        # start=False (PSUM pending-zero rules)
                        nc.tensor.matmul(po[:, 0:512], zrow[0:1, 0:65],
                                         zrow[0:1, 0:512],
                                         start=True, stop=False,
                                         skip_group_check=True)
                    dg = diag_p.tile([128, 128], BF16, tag="dg",
                                     name=f"dg{h}_{kc}")
                    nc.gpsimd.affine_select(
                        out=dg, in_=ptile[:, 0:128],
                        compare_op=mybir.AluOpType.is_ge, fill=0.0,
                        base=0, pattern=[[1, 128]], channel_multiplier=-1)
                    # PV runs one k-chunk behind the scores so the PE never
                    # waits on the exp it just requested: PV(kc-1) reads a
                    # ptile whose exp finished during scores(kc)
                    if state["pv"] is not None:
                        state["pv"]()
                        state["pv"] = None
                    if state["diag"] is not None:
                        state["diag"]()
                        state["diag"] = None

                    def pv_pieces(kcc=kc, ptt=ptile, poo=po, hh=h):
                        # non-diagonal PV: q in [q0+128, S), 512-aligned
                        # pieces; kc==0 pieces outside bank 0 open their
                        # accumulation group (bank 0 was pre-zeroed)
                        a = 128 * kcc + 128
                        while a < S:
                            b_ = min(S, (a // 512 + 1) * 512)
                            nc.tensor.matmul(poo[:, a:b_],
                                             v65[:, kcc, hh, 0:65],
                                             ptt[:, a - 128 * kcc:b_ - 128 * kcc],
                                             start=(kcc == 0 and a >= 512),
                                             stop=False, skip_group_check=True)
                            a = b_

                    state["pv"] = pv_pieces
                    state["diag"] = (
                        lambda kcc=kc, dgg=dg, poo=po, hh=h: nc.tensor.matmul(
                            poo[:, kcc * 128:(kcc + 1) * 128],
                            v65[:, kcc, hh, 0:65], dgg[:, :],
                            start=False, stop=True, skip_group_check=True))
                    for th in fillers.get(kc, ()):
                        th()
                state["pv"]()
                state["pv"] = None
                state["diag"]()
                state["diag"] = None
                state["norm"] = lambda poo=po, hh=h: normalize(hh, poo)

            # ---------- schedule ----------
            # warm the PE clock (p-state ramps over ~3us of busy time)
            # while the input DMAs land
            for w in range(8):
                wt = stp_p.tile([128, 1024], F32, tag="stp", name=f"warm{w}")
                nc.tensor.matmul(wt[:, 0:512], zrow[0:1, 0:128],
                                 zrow[0:1, 0:512], start=True, stop=True,
                                 skip_group_check=True)
            # prefix: head 0's q must be complete (scores read qt[:, q0:S]);
            # its k and v arrive narrowly (kt[:, q0:q0+128] / v65[:, kc]) so
            # later chunks stream in as filler
            for m_, ss_ in ((0, 0), (0, 1), (4, 0), (0, 2), (0, 3)):
                qk_chain(m_, ss_)
            v_chain(0)

            def QK(m, ss):
                return lambda: qk_chain(m, ss)

            def VC(*stts):
                return lambda: [v_chain(s) for s in stts]

            def CP(gc, *stts):
                return lambda: [cproj_chunk(gc, s) for s in stts]

            # filler placement rules: a chain emitted at slot kc is only
            # readable from chunk kc+1 on (reads emitted before writes see
            # stale data).  v65[:, k] is needed at PV(k); kt chunk ss at
            # scores(4*ss); q chunks must be complete before the head starts.
            fillers = {
                0: {0: [VC(1)], 1: [QK(4, 1), VC(2)], 2: [VC(3, 4)],
                    3: [VC(5, 6)], 5: [QK(4, 2), VC(7, 8)], 7: [VC(9, 10)],
                    9: [QK(4, 3), VC(11, 12)], 11: [VC(13, 14)], 13: [VC(15)]},
                1: {1: [QK(1, 0)], 3: [QK(5, 0)], 5: [QK(1, 1)], 7: [QK(5, 1)],
                    9: [QK(1, 2)], 11: [QK(5, 2)], 13: [QK(1, 3), QK(5, 3)]},
                2: {1: [QK(2, 0)], 3: [QK(6, 0)], 5: [QK(2, 1)], 7: [QK(6, 1)]},
                3: {1: [QK(2, 2)], 3: [QK(6, 2)], 5: [QK(2, 3)], 7: [QK(6, 3)]},
                4: {1: [QK(3, 0)], 3: [QK(7, 0)], 5: [QK(3, 1)], 7: [QK(7, 1)],
                    9: [CP(0, 0)], 11: [CP(0, 1)], 13: [CP(0, 2)]},
                5: {1: [QK(3, 2)], 3: [QK(7, 2)], 5: [QK(3, 3)], 7: [QK(7, 3)],
                    9: [CP(0, 3)], 11: [CP(0, 4)], 13: [CP(0, 5)]},
                6: {1: [CP(0, 6)], 3: [CP(0, 7)], 5: [CP(0, 8)], 7: [CP(0, 9)],
                    9: [CP(0, 10)], 11: [CP(0, 11)], 13: [CP(0, 12)]},
                7: {1: [CP(0, 13)], 3: [CP(0, 14)], 5: [CP(0, 15)]},
            }
            for h in range(8):
                head_attention(h, fillers[h])
            state["norm"]()
            # tail: second half of c_proj, PSUM evacuation split between DVE
            # and the now-idle Act engine
            for stt in range(NT):
                cproj_chunk(1, stt, on_act=(stt % 2 == 1))

    nc.compile()
    return nc


def _fp8(a, scale=1.0):
    return (np.asarray(a, np.float32) * scale).astype(ml_dtypes.float8_e4m3)


def _bf16(a):
    return np.asarray(a, np.float32).astype(ml_dtypes.bfloat16)


def prep_core_inputs(hidden_states, position_states, Wq, bq, Wqh, bqh, Wk, bk,
                     Wkh, bkh, Wv, bv, Wvh, bvh, Wp, bp, Wpe, bpe, Wc, bc):
    """Host-side weight folding + per-core staging."""
    f32 = np.float32
    eyeE = np.eye(E, dtype=f32)

    def fold(parity):
        hs = slice(G * parity, G * parity + G)
        csl = slice(512 * parity, 512 * parity + 512)
        mats = {}
        for name, (Wa, ba, Wh, bh, v) in {
            "q": (Wq, bq, Wqh[hs], bqh[hs], 0),
            "k": (Wk, bk, Wkh[hs], bkh[hs], 1),
            "v": (Wv, bv, Wvh[hs], bvh[hs], 2),
        }.items():
            mx = np.einsum("hed,ghd->hegd", Wa, Wh).reshape(E, 512)
            mp = np.einsum("pd,g->pgd", Wp[:, v * D:(v + 1) * D],
                           Wpe[v, 0, hs]).reshape(P, 512)
            bias = (np.einsum("hd,ghd->gd", ba, Wh) + bh
                    + bp[v * D:(v + 1) * D][None, :] * Wpe[v, 0, hs][:, None]
                    + bpe[hs][:, None]).reshape(512)
            C = np.zeros((NPAIR * 256, 512), f32)
            C[:E] = mx - eyeE[:, csl]
            C[E:E + P] = mp
            C[E + P] = bias
            mats[name] = C
        cqk = np.concatenate([mats["q"], mats["k"]], axis=1)     # [1280, 1024]
        cqk8 = _fp8(cqk.reshape(NPAIR, 2, 128, 1024).transpose(0, 2, 1, 3), CSCALE)
        cv8 = _fp8(mats["v"].reshape(NPAIR, 2, 128, 512).transpose(0, 2, 1, 3), CSCALE)
        wc = Wc.reshape(H, D, E)[hs].reshape(512, E).reshape(4, 128, E)
        return (np.ascontiguousarray(cqk8), np.ascontiguousarray(cv8),
                np.ascontiguousarray(_bf16(wc)))

    per_parity = [fold(0), fold(1)]

    in_maps = []
    for c in range(NCORE):
        b, parity = c // 2, c % 2
        csl = slice(512 * parity, 512 * parity + 512)
        xaug = np.zeros((NPAIR * 256, S), f32)
        xaug[:E] = hidden_states[b].T
        xaug[E:E + P] = position_states[b].T
        xaug[E + P] = 1.0
        x8 = _fp8(xaug)
        dx = xaug[:E] - x8[:E].astype(f32)
        xt8 = np.ascontiguousarray(
            x8.reshape(NPAIR, 2, 128, S).transpose(0, 2, 1, 3))
        dxt8 = np.ascontiguousarray(
            _fp8(dx).reshape(4, 2, 128, S).transpose(0, 2, 1, 3))
        xtid = np.ascontiguousarray(
            _bf16(hidden_states[b].T[csl]).reshape(4, 128, S))
        xnat = np.ascontiguousarray(
            _bf16(hidden_states[b][:, csl]).reshape(NT, 128, 512))
        cqk8, cv8, wc = per_parity[parity]
        in_maps.append({"cqk8": cqk8, "cv8": cv8, "xt8": xt8, "dxt8": dxt8,
                        "xtid": xtid, "xnat": xnat, "wc16": wc,
                        "onesb": _bf16(np.ones((128, 128), f32)),
                        "onesr": np.ones((1, 64), f32),
                        "zerob": _bf16(np.zeros((1, 512), f32))})
    return in_maps


_NC_CACHE = {}


def get_nc():
    if "nc" not in _NC_CACHE:
        _NC_CACHE["nc"] = build_nc()
    return _NC_CACHE["nc"]


def assemble(results, bc):
    outs = []
    for b in range(B):
        acc = np.zeros((S, E), np.float32)
        for p in range(2):
            acc += results[2 * b + p]["outp"].astype(np.float32).sum(axis=0)
        outs.append(acc + bc)
    return np.stack(outs).astype(np.float32)


def kernel(**inputs):
    nc = get_nc()
    in_maps = prep_core_inputs(**inputs)
    res = run_bass_kernel_spmd(nc, in_maps, list(range(NCORE)))
    return assemble(res.results, inputs["bc"])


# revision 16
# speedup vs baseline: 1.1676x; 1.0108x over previous
"""Trainium2 Bass kernel for nn_Attention_77025943487081.

Sharding: batch (4) data-parallel x 2-way head tensor-parallel over 8 cores.
Core c handles batch c//2 and heads [8*(c%2), 8*(c%2)+8). Each core emits 4
partial c_proj outputs (one per head-pair, bf16); the host sums the 8 partials
per batch and adds the c_proj bias.

Numerics (validated in numpy against the f32 reference, rel err ~8e-3 vs the
2e-2 gate):
  - The folded per-head+cross-head projection matrices are I + C with C at
    0.02 scale.  q/k/v = bf16(x_slice) + (x8 + dx8) @ C8 / 32 where x8/dx8
    are fp8e4m3 value+residual and C8 = fp8(32*C) (the 32x pre-scale keeps
    C's entries out of fp8's subnormal range). The correction matmuls run in
    fp8 DoubleRow mode (256-deep contraction, 0.5 cycles/row).
  - Scores/PV/c_proj operands are bf16, accumulation always f32 PSUM.
  - The 1/sqrt(D) score scale is applied inside the softmax exp activation
    (out = exp(in * 0.125)), so q is staged unscaled.

Attention per head: scores^T [k, q] per 128-wide k-chunk -> exp on Act ->
PV in [d, q] orientation (po65[65, S] accumulator; row 64 collects softmax
denominators via a ones row appended to V). Wide PV matmuls (<=512 q
columns) keep the PE instruction count low — per-instruction semaphore and
issue overhead (~50-100 ns) is what kills many-small-matmul schedules. The
diagonal (causally masked) block's PV is deferred one k-chunk so the Pool
affine_select round-trip hides behind the next chunk's scores.

Schedule: the Act engine paces each head (~19 us of softmax exp vs ~16 us
of PE work), so the remaining phase-1 chains and the previous pair's
partial c_proj run as filler inside later heads' k-chunk loops. Each head's
normalization (1/den broadcast via a ones-column matmul) is deferred into
the next head's first k-chunk. The final pair's c_proj evacuates PSUM on
both DVE and Act (Act is idle by then).
"""

import numpy as np
import ml_dtypes
from contextlib import ExitStack

import concourse.bass as bass
import concourse.tile as tile
from concourse import bacc, mybir
from concourse.bass_utils import run_bass_kernel_spmd

F32 = mybir.dt.float32
F32R = mybir.dt.float32r
BF16 = mybir.dt.bfloat16
FP8 = mybir.dt.float8e4
DR = mybir.MatmulPerfMode.DoubleRow
ACT_EXP = mybir.ActivationFunctionType.Exp
ACT_COPY = mybir.ActivationFunctionType.Copy
MULT = mybir.AluOpType.mult
ADD = mybir.AluOpType.add

B, S, E, H, D, P = 4, 2048, 1024, 16, 64, 64
G = 8            # heads per core
NCORE = 8
NT = S // 128    # 16 sequence tiles
NPAIR = 5        # fp8 DoubleRow contraction pairs: 1280 = 5*256 rows (1089 used)
CSCALE = 32.0    # fp8 pre-scale on the correction matrices
INV_CS = 1.0 / CSCALE


def build_nc():
    nc = bacc.Bacc("TRN2", target_bir_lowering=False, debug=False, num_devices=NCORE)
    cqk8 = nc.dram_tensor("cqk8", [128, NPAIR, 2, 1024], FP8, kind="ExternalInput").ap()
    cv8 = nc.dram_tensor("cv8", [128, NPAIR, 2, 512], FP8, kind="ExternalInput").ap()
    xt8 = nc.dram_tensor("xt8", [128, NPAIR, 2, S], FP8, kind="ExternalInput").ap()
    dxt8 = nc.dram_tensor("dxt8", [128, 4, 2, S], FP8, kind="ExternalInput").ap()
    xtid = nc.dram_tensor("xtid", [128, 4, S], BF16, kind="ExternalInput").ap()
    xnat = nc.dram_tensor("xnat", [128, NT, 512], BF16, kind="ExternalInput").ap()
    wc16 = nc.dram_tensor("wc16", [128, 4, E], BF16, kind="ExternalInput").ap()
    onesb = nc.dram_tensor("onesb", [128, 128], BF16, kind="ExternalInput").ap()
    onesr = nc.dram_tensor("onesr", [1, 64], F32R, kind="ExternalInput").ap()
    zerob = nc.dram_tensor("zerob", [1, 512], BF16, kind="ExternalInput").ap()
    outp = nc.dram_tensor("outp", [2, S, E], BF16, kind="ExternalOutput").ap()

    with nc.allow_low_precision("bf16/fp8 staged operands; f32 PSUM accumulation"), \
         tile.TileContext(nc) as tc, ExitStack() as top:
        const_p = top.enter_context(tc.tile_pool(name="const", bufs=1))
        qk_p = top.enter_context(tc.tile_pool(name="qkt", bufs=1))
        vaug_p = top.enter_context(tc.tile_pool(name="vaug", bufs=1))
        oT_p = top.enter_context(tc.tile_pool(name="oT", bufs=1))

        cqk_sb = const_p.tile([128, NPAIR, 2, 1024], FP8)
        cv_sb = const_p.tile([128, NPAIR, 2, 512], FP8)
        xt_sb = const_p.tile([128, NPAIR, 2, S], FP8)
        dxt_sb = const_p.tile([128, 4, 2, S], FP8)
        xtid_sb = const_p.tile([128, 4, S], BF16)
        xnat_sb = const_p.tile([128, NT, 512], BF16)
        wc_sb = const_p.tile([128, 4, E], BF16)
        zrow = const_p.tile([1, 512], BF16)
        ones64 = const_p.tile([1, 64], F32R)

        qkt = [qk_p.tile([128, S], BF16, name=f"qkt{m}") for m in range(8)]
        v65 = vaug_p.tile([128, NT, G, 66], BF16)   # col 64 = ones (denominator)
        oT = oT_p.tile([128, 4, S], BF16)

        # --- DMAs, startup-critical first -------------------------------
        # each DMA pays ~625ns serialized HWDGE overhead + ~900ns sem
        # propagation, so transfers are consolidated (DRAM layouts mirror the
        # SBUF tiles, partition-major)
        nc.sync.dma_start(out=zrow, in_=zerob)
        # stage 1: what the first chains (m0/m4, all pairs) + head-0 kc0 need
        nc.sync.dma_start(out=cqk_sb[:, :, :, 0:128], in_=cqk8[:, :, :, 0:128])
        nc.sync.dma_start(out=cqk_sb[:, :, :, 512:640], in_=cqk8[:, :, :, 512:640])
        nc.sync.dma_start(out=xt_sb[:, :, :, 0:1024], in_=xt8[:, :, :, 0:1024])
        nc.sync.dma_start(out=dxt_sb[:, :, :, 0:1024], in_=dxt8[:, :, :, 0:1024])
        nc.sync.dma_start(out=xtid_sb[:, 0, 0:1024], in_=xtid[:, 0, 0:1024])
        # stage 2: second halves, v-path, remaining weights
        nc.sync.dma_start(out=xt_sb[:, :, :, 1024:2048], in_=xt8[:, :, :, 1024:2048])
        nc.sync.dma_start(out=dxt_sb[:, :, :, 1024:2048],
                          in_=dxt8[:, :, :, 1024:2048])
        nc.sync.dma_start(out=xtid_sb[:, 0, 1024:2048], in_=xtid[:, 0, 1024:2048])
        nc.sync.dma_start(out=cv_sb, in_=cv8)
        nc.sync.dma_start(out=xnat_sb[:, 0:8], in_=xnat[:, 0:8])
        nc.sync.dma_start(out=xnat_sb[:, 8:NT], in_=xnat[:, 8:NT])
        nc.sync.dma_start(out=cqk_sb[:, :, :, 128:512], in_=cqk8[:, :, :, 128:512])
        nc.sync.dma_start(out=cqk_sb[:, :, :, 640:1024],
                          in_=cqk8[:, :, :, 640:1024])
        nc.sync.dma_start(out=xtid_sb[:, 1:4], in_=xtid[:, 1:4])
        nc.sync.dma_start(out=wc_sb, in_=wc16)
        nc.sync.dma_start(out=ones64, in_=onesr)
        for col in (64, 65):
            nc.sync.dma_start(
                out=v65[:, :, :, col:col + 1],
                in_=onesb[:, 0:128].rearrange("p (a b c) -> p a b c", a=16, b=8, c=1))

        with tc.tile_pool(name="stp", bufs=2, space="PSUM") as stp_p, \
             tc.tile_pool(name="pop", bufs=1, space="PSUM") as po_p, \
             tc.tile_pool(name="pt", bufs=2) as pt_p, \
             tc.tile_pool(name="diag", bufs=2) as diag_p, \
             tc.tile_pool(name="bcst", bufs=2) as bcst_p, \
             tc.tile_pool(name="rcpp", bufs=2) as rcp_p, \
             tc.tile_pool(name="ost", bufs=3) as ost_p:

            # ---------- phase-1 building blocks ----------
            def qk_chain(m, ss):
                """qkt[m][:, ss*512:+512] = identity x-slice + fp8 correction."""
                ps = stp_p.tile([128, 1024], F32, tag="stp", name=f"qkps{m}_{ss}")
                side = 0 if m < 4 else 512
                col0 = side + (m % 4) * 128
                xsl = slice(ss * 512, (ss + 1) * 512)
                for pr in range(NPAIR):
                    nc.tensor.matmul(ps[:, 0:512],
                                     cqk_sb[:, pr, :, col0:col0 + 128],
                                     xt_sb[:, pr, :, xsl],
                                     start=(pr == 0), stop=False, perf_mode=DR)
                for pr in range(4):
                    nc.tensor.matmul(ps[:, 0:512],
                                     cqk_sb[:, pr, :, col0:col0 + 128],
                                     dxt_sb[:, pr, :, xsl],
                                     start=False, stop=(pr == 3), perf_mode=DR)
                nc.vector.scalar_tensor_tensor(
                    out=qkt[m][:, xsl], in0=ps[:, 0:512], scalar=INV_CS,
                    in1=xtid_sb[:, m % 4, xsl], op0=MULT, op1=ADD)

            def v_chain(stt):
                pv = stp_p.tile([128, 1024], F32, tag="stp", name=f"vps{stt}")
                for pr in range(NPAIR):
                    nc.tensor.matmul(pv[:, 0:512],
                                     xt_sb[:, pr, :, stt * 128:(stt + 1) * 128],
                                     cv_sb[:, pr, :, :],
                                     start=(pr == 0), stop=(pr == NPAIR - 1),
                                     perf_mode=DR)
                nc.vector.scalar_tensor_tensor(
                    out=v65[:, stt, :, 0:64],
                    in0=pv[:, 0:512].rearrange("p (g d) -> p g d", g=G),
                    scalar=INV_CS,
                    in1=xnat_sb[:, stt].rearrange("p (g d) -> p g d", g=G),
                    op0=MULT, op1=ADD)

            def cproj_chunk(part, stt, on_act=False):
                """one 128-row slab of the half c_proj (head-pairs 2p, 2p+1)"""
                pc = stp_p.tile([128, 1024], F32, tag="stp", name=f"pc{part}_{stt}")
                for gi, gc in enumerate((2 * part, 2 * part + 1)):
                    for ee in range(2):
                        nc.tensor.matmul(pc[:, ee * 512:(ee + 1) * 512],
                                         oT[:, gc, stt * 128:(stt + 1) * 128],
                                         wc_sb[:, gc, ee * 512:(ee + 1) * 512],
                                         start=(gi == 0), stop=(gi == 1),
                                         skip_group_check=True)
                ost = ost_p.tile([128, E], BF16, tag="ost", name=f"ost{part}_{stt}")
                if on_act:
                    nc.scalar.activation(ost[:, :], pc[:, :], ACT_COPY)
                else:
                    nc.vector.tensor_copy(ost[:, :], pc[:, :])
                nc.sync.dma_start(out=outp[part, stt * 128:(stt + 1) * 128, :],
                                  in_=ost[:, :])

            # ---------- phase-2 per-head attention ----------
            state = {"norm": None, "diag": None, "pv": None}

            def normalize(h, po):
                """oT[d-half, pair, :] = po[0:64, :] / po[64, :] (bf16).
                The per-q reciprocal is broadcast across partitions by the
                otherwise-idle GPSIMD engine, keeping the PE and the scores
                PSUM pool out of it."""
                m, half = h // 2, h % 2
                rcp = rcp_p.tile([1, S], F32R, tag="rcp", name=f"rcp{h}")
                nc.vector.reciprocal(rcp, po[64:65, :])
                bcst = bcst_p.tile([64, S], F32R, tag="bcst", name=f"bcst{h}")
                nc.gpsimd.partition_broadcast(bcst[:, :], rcp[0:1, :])
                nc.vector.tensor_mul(oT[64 * half:64 * half + 64, m, :],
                                     po[0:64, :], bcst[:, :])

            def head_attention(h, fillers):
                """fillers: dict kc -> list of thunks emitted at that k-chunk."""
                m, half = h // 2, h % 2
                qt = qkt[m][64 * half:64 * half + 64, :]
                kt = qkt[4 + m][64 * half:64 * half + 64, :]
                po = None
                for kc in range(NT):
                    q0 = 128 * kc
                    ptile = pt_p.tile([128, 2048], BF16, tag="pt",
                                      name=f"pt{h}_{kc}")
                    for c0 in range(q0, S, 1024):
                        cw = min(1024, S - c0)
                        st = stp_p.tile([128, 1024], F32, tag="stp",
                                        name=f"st{h}_{kc}_{c0}")
                        for u0 in range(c0, c0 + cw, 512):
                            uw = min(512, c0 + cw - u0)
                            nc.tensor.matmul(st[:, u0 - c0:u0 - c0 + uw],
                                             kt[:, q0:q0 + 128], qt[:, u0:u0 + uw],
                                             start=True, stop=True)
                        nc.scalar.activation(ptile[:, c0 - q0:c0 - q0 + cw],
                                             st[:, 0:cw], ACT_EXP, scale=0.125)
                    if kc == 0:
                        # previous head's deferred normalization must be
                        # emitted before its po slot is reallocated below
                        if state["norm"] is not None:
                            state["norm"]()
                            state["norm"] = None
                        po = po_p.tile([128, S], F32, tag="po", name=f"po{h}")
                        # zero bank 0 so every PV there can accumulate with
                        # start=False (PSUM pending-zero rules)
                        nc.tensor.matmul(po[0:65, 0:512], zrow[0:1, 0:65],
                                         zrow[0:1, 0:512],
                                         start=True, stop=False,
                                         skip_group_check=True)
                    dg = diag_p.tile([128, 128], BF16, tag="dg",
                                     name=f"dg{h}_{kc}")
                    nc.gpsimd.affine_select(
                        out=dg, in_=ptile[:, 0:128],
                        compare_op=mybir.AluOpType.is_ge, fill=0.0,
                        base=0, pattern=[[1, 128]], channel_multiplier=-1)
                    # PV runs one k-chunk behind the scores so the PE never
                    # waits on the exp it just requested: PV(kc-1) reads a
                    # ptile whose exp finished during scores(kc)
                    if state["pv"] is not None:
                        state["pv"]()
                        state["pv"] = None
                    if state["diag"] is not None:
                        state["diag"]()
                        state["diag"] = None

                    def pv_pieces(kcc=kc, ptt=ptile, poo=po, hh=h):
                        # non-diagonal PV: q in [q0+128, S), 512-aligned
                        # pieces; kc==0 pieces outside bank 0 open their
                        # accumulation group (bank 0 was pre-zeroed)
                        a = 128 * kcc + 128
                        while a < S:
                            b_ = min(S, (a // 512 + 1) * 512)
                            nc.tensor.matmul(poo[0:65, a:b_],
                                             v65[:, kcc, hh, 0:65],
                                             ptt[:, a - 128 * kcc:b_ - 128 * kcc],
                                             start=(kcc == 0 and a >= 512),
                                             stop=False, skip_group_check=True)
                            a = b_

                    state["pv"] = pv_pieces
                    state["diag"] = (
                        lambda kcc=kc, dgg=dg, poo=po, hh=h: nc.tensor.matmul(
                            poo[0:65, kcc * 128:(kcc + 1) * 128],
                            v65[:, kcc, hh, 0:65], dgg[:, :],
                            start=False, stop=True, skip_group_check=True))
                    for th in fillers.get(kc, ()):
                        th()
                state["pv"]()
                state["pv"] = None
                state["diag"]()
                state["diag"] = None
                state["norm"] = lambda poo=po, hh=h: normalize(hh, poo)

            # ---------- schedule ----------
            # warm the PE clock (p-state ramps over ~3us of busy time)
            # while the input DMAs land
            for w in range(8):
                wt = stp_p.tile([128, 1024], F32, tag="stp", name=f"warm{w}")
                nc.tensor.matmul(wt[:, 0:512], zrow[0:1, 0:128],
                                 zrow[0:1, 0:512], start=True, stop=True,
                                 skip_group_check=True)
            # prefix: head 0's q must be complete (scores read qt[:, q0:S]);
            # its k and v arrive narrowly (kt[:, q0:q0+128] / v65[:, kc]) so
            # later chunks stream in as filler
            for m_, ss_ in ((0, 0), (0, 1), (4, 0), (0, 2), (0, 3)):
                qk_chain(m_, ss_)
            v_chain(0)

            def QK(m, ss):
                return lambda: qk_chain(m, ss)

            def VC(*stts):
                return lambda: [v_chain(s) for s in stts]

            def CP(gc, *stts):
                return lambda: [cproj_chunk(gc, s) for s in stts]

            # filler placement rules: a chain emitted at slot kc is only
            # readable from chunk kc+1 on (reads emitted before writes see
            # stale data).  v65[:, k] is needed at PV(k); kt chunk ss at
            # scores(4*ss); q chunks must be complete before the head starts.
            fillers = {
                0: {0: [VC(1)], 1: [QK(4, 1), VC(2)], 2: [VC(3, 4)],
                    3: [VC(5, 6)], 5: [QK(4, 2), VC(7, 8)], 7: [VC(9, 10)],
                    9: [QK(4, 3), VC(11, 12)], 11: [VC(13, 14)], 13: [VC(15)]},
                1: {1: [QK(1, 0)], 3: [QK(5, 0)], 5: [QK(1, 1)], 7: [QK(5, 1)],
                    9: [QK(1, 2)], 11: [QK(5, 2)], 13: [QK(1, 3), QK(5, 3)]},
                2: {1: [QK(2, 0)], 3: [QK(6, 0)], 5: [QK(2, 1)], 7: [QK(6, 1)]},
                3: {1: [QK(2, 2)], 3: [QK(6, 2)], 5: [QK(2, 3)], 7: [QK(6, 3)]},
                4: {1: [QK(3, 0)], 3: [QK(7, 0)], 5: [QK(3, 1)], 7: [QK(7, 1)],
                    9: [CP(0, 0)], 11: [CP(0, 1)], 13: [CP(0, 2)]},
                5: {1: [QK(3, 2)], 3: [QK(7, 2)], 5: [QK(3, 3)], 7: [QK(7, 3)],
                    9: [CP(0, 3)], 11: [CP(0, 4)], 13: [CP(0, 5)]},
                6: {1: [CP(0, 6)], 3: [CP(0, 7)], 5: [CP(0, 8)], 7: [CP(0, 9)],
                    9: [CP(0, 10)], 11: [CP(0, 11)], 13: [CP(0, 12)]},
                7: {1: [CP(0, 13)], 3: [CP(0, 14)], 5: [CP(0, 15)]},
            }
            for h in range(8):
                head_attention(h, fillers[h])
            state["norm"]()
            # tail: second half of c_proj, PSUM evacuation split between DVE
            # and the now-idle Act engine
            for stt in range(NT):
                cproj_chunk(1, stt, on_act=(stt % 2 == 1))

    nc.compile()
    return nc


def _fp8(a, scale=1.0):
    return (np.asarray(a, np.float32) * scale).astype(ml_dtypes.float8_e4m3)


def _bf16(a):
    return np.asarray(a, np.float32).astype(ml_dtypes.bfloat16)


def prep_core_inputs(hidden_states, position_states, Wq, bq, Wqh, bqh, Wk, bk,
                     Wkh, bkh, Wv, bv, Wvh, bvh, Wp, bp, Wpe, bpe, Wc, bc):
    """Host-side weight folding + per-core staging."""
    f32 = np.float32
    eyeE = np.eye(E, dtype=f32)

    def fold(parity):
        hs = slice(G * parity, G * parity + G)
        csl = slice(512 * parity, 512 * parity + 512)
        mats = {}
        for name, (Wa, ba, Wh, bh, v) in {
            "q": (Wq, bq, Wqh[hs], bqh[hs], 0),
            "k": (Wk, bk, Wkh[hs], bkh[hs], 1),
            "v": (Wv, bv, Wvh[hs], bvh[hs], 2),
        }.items():
            mx = np.einsum("hed,ghd->hegd", Wa, Wh).reshape(E, 512)
            mp = np.einsum("pd,g->pgd", Wp[:, v * D:(v + 1) * D],
                           Wpe[v, 0, hs]).reshape(P, 512)
            bias = (np.einsum("hd,ghd->gd", ba, Wh) + bh
                    + bp[v * D:(v + 1) * D][None, :] * Wpe[v, 0, hs][:, None]
                    + bpe[hs][:, None]).reshape(512)
            C = np.zeros((NPAIR * 256, 512), f32)
            C[:E] = mx - eyeE[:, csl]
            C[E:E + P] = mp
            C[E + P] = bias
            mats[name] = C
        cqk = np.concatenate([mats["q"], mats["k"]], axis=1)     # [1280, 1024]
        cqk8 = _fp8(cqk.reshape(NPAIR, 2, 128, 1024).transpose(2, 0, 1, 3), CSCALE)
        cv8 = _fp8(mats["v"].reshape(NPAIR, 2, 128, 512).transpose(2, 0, 1, 3), CSCALE)
        wc = Wc.reshape(H, D, E)[hs].reshape(512, E).reshape(4, 128, E).transpose(1, 0, 2)
        return (np.ascontiguousarray(cqk8), np.ascontiguousarray(cv8),
                np.ascontiguousarray(_bf16(wc)))

    per_parity = [fold(0), fold(1)]

    in_maps = []
    for c in range(NCORE):
        b, parity = c // 2, c % 2
        csl = slice(512 * parity, 512 * parity + 512)
        xaug = np.zeros((NPAIR * 256, S), f32)
        xaug[:E] = hidden_states[b].T
        xaug[E:E + P] = position_states[b].T
        xaug[E + P] = 1.0
        x8 = _fp8(xaug)
        dx = xaug[:E] - x8[:E].astype(f32)
        xt8 = np.ascontiguousarray(
            x8.reshape(NPAIR, 2, 128, S).transpose(2, 0, 1, 3))
        dxt8 = np.ascontiguousarray(
            _fp8(dx).reshape(4, 2, 128, S).transpose(2, 0, 1, 3))
        xtid = np.ascontiguousarray(
            _bf16(hidden_states[b].T[csl]).reshape(4, 128, S).transpose(1, 0, 2))
        xnat = np.ascontiguousarray(
            _bf16(hidden_states[b][:, csl]).reshape(NT, 128, 512).transpose(1, 0, 2))
        cqk8, cv8, wc = per_parity[parity]
        in_maps.append({"cqk8": cqk8, "cv8": cv8, "xt8": xt8, "dxt8": dxt8,
                        "xtid": xtid, "xnat": xnat, "wc16": wc,
                        "onesb": _bf16(np.ones((128, 128), f32)),
                        "onesr": np.ones((1, 64), f32),
                        "zerob": _bf16(np.zeros((1, 512), f32))})
    return in_maps


_NC_CACHE = {}


def get_nc():
    if "nc" not in _NC_CACHE:
        _NC_CACHE["nc"] = build_nc()
    return _NC_CACHE["nc"]


def assemble(results, bc):
    outs = []
    for b in range(B):
        acc = np.zeros((S, E), np.float32)
        for p in range(2):
            acc += results[2 * b + p]["outp"].astype(np.float32).sum(axis=0)
        outs.append(acc + bc)
    return np.stack(outs).astype(np.float32)


def kernel(**inputs):
    nc = get_nc()
    in_maps = prep_core_inputs(**inputs)
    res = run_bass_kernel_spmd(nc, in_maps, list(range(NCORE)))
    return assemble(res.results, inputs["bc"])


# revision 17
# speedup vs baseline: 1.2400x; 1.0620x over previous
"""Trainium2 Bass kernel for nn_Attention_77025943487081.

Sharding: batch (4) data-parallel x 2-way head tensor-parallel over 8 cores.
Core c handles batch c//2 and heads [8*(c%2), 8*(c%2)+8). Each core emits 4
partial c_proj outputs (one per head-pair, bf16); the host sums the 8 partials
per batch and adds the c_proj bias.

Numerics (validated in numpy against the f32 reference, rel err ~8e-3 vs the
2e-2 gate):
  - The folded per-head+cross-head projection matrices are I + C with C at
    0.02 scale.  q/k/v = bf16(x_slice) + (x8 + dx8) @ C8 / 32 where x8/dx8
    are fp8e4m3 value+residual and C8 = fp8(32*C) (the 32x pre-scale keeps
    C's entries out of fp8's subnormal range). The correction matmuls run in
    fp8 DoubleRow mode (256-deep contraction, 0.5 cycles/row).
  - Scores/PV/c_proj operands are bf16, accumulation always f32 PSUM.
  - The 1/sqrt(D) score scale is applied inside the softmax exp activation
    (out = exp(in * 0.125)), so q is staged unscaled.

Attention per head: scores^T [k, q] per 128-wide k-chunk -> exp on Act ->
PV in [d, q] orientation (po65[65, S] accumulator; row 64 collects softmax
denominators via a ones row appended to V). Wide PV matmuls (<=512 q
columns) keep the PE instruction count low — per-instruction semaphore and
issue overhead (~50-100 ns) is what kills many-small-matmul schedules. The
diagonal (causally masked) block's PV is deferred one k-chunk so the Pool
affine_select round-trip hides behind the next chunk's scores.

Schedule: the Act engine paces each head (~19 us of softmax exp vs ~16 us
of PE work), so the remaining phase-1 chains and the previous pair's
partial c_proj run as filler inside later heads' k-chunk loops. Each head's
normalization (1/den broadcast via a ones-column matmul) is deferred into
the next head's first k-chunk. The final pair's c_proj evacuates PSUM on
both DVE and Act (Act is idle by then).
"""

import numpy as np
import ml_dtypes
from contextlib import ExitStack

import concourse.bass as bass
import concourse.tile as tile
from concourse import bacc, mybir
from concourse.bass_utils import run_bass_kernel_spmd

F32 = mybir.dt.float32
F32R = mybir.dt.float32r
BF16 = mybir.dt.bfloat16
FP8 = mybir.dt.float8e4
DR = mybir.MatmulPerfMode.DoubleRow
ACT_EXP = mybir.ActivationFunctionType.Exp
ACT_COPY = mybir.ActivationFunctionType.Copy
MULT = mybir.AluOpType.mult
ADD = mybir.AluOpType.add

B, S, E, H, D, P = 4, 2048, 1024, 16, 64, 64
G = 8            # heads per core
NCORE = 8
NT = S // 128    # 16 sequence tiles
NPAIR = 5        # fp8 DoubleRow contraction pairs: 1280 = 5*256 rows (1089 used)
CSCALE = 32.0    # fp8 pre-scale on the correction matrices
INV_CS = 1.0 / CSCALE


def build_nc():
    nc = bacc.Bacc("TRN2", target_bir_lowering=False, debug=False, num_devices=NCORE)
    cqk8 = nc.dram_tensor("cqk8", [128, NPAIR, 2, 1024], FP8, kind="ExternalInput").ap()
    cv8 = nc.dram_tensor("cv8", [128, NPAIR, 2, 512], FP8, kind="ExternalInput").ap()
    xt8 = nc.dram_tensor("xt8", [128, NPAIR, 2, S], FP8, kind="ExternalInput").ap()
    dxt8 = nc.dram_tensor("dxt8", [128, 4, 2, S], FP8, kind="ExternalInput").ap()
    xtid = nc.dram_tensor("xtid", [128, 4, S], BF16, kind="ExternalInput").ap()
    xnat = nc.dram_tensor("xnat", [128, NT, 512], BF16, kind="ExternalInput").ap()
    wc16 = nc.dram_tensor("wc16", [128, 4, E], BF16, kind="ExternalInput").ap()
    onesb = nc.dram_tensor("onesb", [128, 128], BF16, kind="ExternalInput").ap()
    zerob = nc.dram_tensor("zerob", [1, 512], BF16, kind="ExternalInput").ap()
    outp = nc.dram_tensor("outp", [2, S, E], BF16, kind="ExternalOutput").ap()

    with nc.allow_low_precision("bf16/fp8 staged operands; f32 PSUM accumulation"), \
         tile.TileContext(nc) as tc, ExitStack() as top:
        const_p = top.enter_context(tc.tile_pool(name="const", bufs=1))
        qk_p = top.enter_context(tc.tile_pool(name="qkt", bufs=1))
        vaug_p = top.enter_context(tc.tile_pool(name="vaug", bufs=1))
        oT_p = top.enter_context(tc.tile_pool(name="oT", bufs=1))

        cqk_sb = const_p.tile([128, NPAIR, 2, 1024], FP8)
        cv_sb = const_p.tile([128, NPAIR, 2, 512], FP8)
        xt_sb = const_p.tile([128, NPAIR, 2, S], FP8)
        dxt_sb = const_p.tile([128, 4, 2, S], FP8)
        xtid_sb = const_p.tile([128, 4, S], BF16)
        xnat_sb = const_p.tile([128, NT, 512], BF16)
        wc_sb = const_p.tile([128, 4, E], BF16)
        zrow = const_p.tile([1, 512], BF16)

        qkt = [qk_p.tile([128, S], BF16, name=f"qkt{m}") for m in range(8)]
        v65 = vaug_p.tile([128, NT, G, 66], BF16)   # col 64 = ones (denominator)
        oT = oT_p.tile([128, 4, S], BF16)

        # --- DMAs, startup-critical first -------------------------------
        # each DMA pays ~625ns serialized HWDGE overhead + ~900ns sem
        # propagation, so transfers are consolidated (DRAM layouts mirror the
        # SBUF tiles, partition-major)
        nc.sync.dma_start(out=zrow, in_=zerob)
        for col in (64, 65):
            nc.sync.dma_start(
                out=v65[:, :, :, col:col + 1],
                in_=onesb[:, 0:128].rearrange("p (a b c) -> p a b c", a=16, b=8, c=1))
        # stage 1: what the first chains (m0/m4, all pairs) + head-0 kc0 need
        nc.sync.dma_start(out=cqk_sb[:, :, :, 0:128], in_=cqk8[:, :, :, 0:128])
        nc.sync.dma_start(out=cqk_sb[:, :, :, 512:640], in_=cqk8[:, :, :, 512:640])
        nc.sync.dma_start(out=xt_sb[:, :, :, 0:1024], in_=xt8[:, :, :, 0:1024])
        nc.sync.dma_start(out=dxt_sb[:, :, :, 0:1024], in_=dxt8[:, :, :, 0:1024])
        nc.sync.dma_start(out=xtid_sb[:, 0, 0:1024], in_=xtid[:, 0, 0:1024])
        # stage 2: second halves, v-path, remaining weights
        nc.sync.dma_start(out=xt_sb[:, :, :, 1024:2048], in_=xt8[:, :, :, 1024:2048])
        nc.sync.dma_start(out=dxt_sb[:, :, :, 1024:2048],
                          in_=dxt8[:, :, :, 1024:2048])
        nc.sync.dma_start(out=xtid_sb[:, 0, 1024:2048], in_=xtid[:, 0, 1024:2048])
        nc.sync.dma_start(out=cv_sb, in_=cv8)
        nc.sync.dma_start(out=xnat_sb[:, 0:8], in_=xnat[:, 0:8])
        nc.sync.dma_start(out=xnat_sb[:, 8:NT], in_=xnat[:, 8:NT])
        nc.sync.dma_start(out=cqk_sb[:, :, :, 128:512], in_=cqk8[:, :, :, 128:512])
        nc.sync.dma_start(out=cqk_sb[:, :, :, 640:1024],
                          in_=cqk8[:, :, :, 640:1024])
        nc.sync.dma_start(out=xtid_sb[:, 1:4], in_=xtid[:, 1:4])
        nc.sync.dma_start(out=wc_sb, in_=wc16)

        with tc.tile_pool(name="stp", bufs=2, space="PSUM") as stp_p, \
             tc.tile_pool(name="pop", bufs=1, space="PSUM") as po_p, \
             tc.tile_pool(name="pt", bufs=2) as pt_p, \
             tc.tile_pool(name="diag", bufs=2) as diag_p, \
             tc.tile_pool(name="bcst", bufs=2) as bcst_p, \
             tc.tile_pool(name="rcpp", bufs=2) as rcp_p, \
             tc.tile_pool(name="ost", bufs=3) as ost_p:

            # ---------- phase-1 building blocks ----------
            def qk_chain(m, ss):
                """qkt[m][:, ss*512:+512] = identity x-slice + fp8 correction."""
                ps = stp_p.tile([128, 1024], F32, tag="stp", name=f"qkps{m}_{ss}")
                side = 0 if m < 4 else 512
                col0 = side + (m % 4) * 128
                xsl = slice(ss * 512, (ss + 1) * 512)
                for pr in range(NPAIR):
                    nc.tensor.matmul(ps[:, 0:512],
                                     cqk_sb[:, pr, :, col0:col0 + 128],
                                     xt_sb[:, pr, :, xsl],
                                     start=(pr == 0), stop=False, perf_mode=DR)
                for pr in range(4):
                    nc.tensor.matmul(ps[:, 0:512],
                                     cqk_sb[:, pr, :, col0:col0 + 128],
                                     dxt_sb[:, pr, :, xsl],
                                     start=False, stop=(pr == 3), perf_mode=DR)
                nc.vector.scalar_tensor_tensor(
                    out=qkt[m][:, xsl], in0=ps[:, 0:512], scalar=INV_CS,
                    in1=xtid_sb[:, m % 4, xsl], op0=MULT, op1=ADD)

            def v_chain(stt):
                pv = stp_p.tile([128, 1024], F32, tag="stp", name=f"vps{stt}")
                for pr in range(NPAIR):
                    nc.tensor.matmul(pv[:, 0:512],
                                     xt_sb[:, pr, :, stt * 128:(stt + 1) * 128],
                                     cv_sb[:, pr, :, :],
                                     start=(pr == 0), stop=(pr == NPAIR - 1),
                                     perf_mode=DR)
                nc.vector.scalar_tensor_tensor(
                    out=v65[:, stt, :, 0:64],
                    in0=pv[:, 0:512].rearrange("p (g d) -> p g d", g=G),
                    scalar=INV_CS,
                    in1=xnat_sb[:, stt].rearrange("p (g d) -> p g d", g=G),
                    op0=MULT, op1=ADD)

            def cproj_chunk(part, stt, on_act=False):
                """one 128-row slab of the half c_proj (head-pairs 2p, 2p+1)"""
                pc = stp_p.tile([128, 1024], F32, tag="stp", name=f"pc{part}_{stt}")
                for gi, gc in enumerate((2 * part, 2 * part + 1)):
                    for ee in range(2):
                        nc.tensor.matmul(pc[:, ee * 512:(ee + 1) * 512],
                                         oT[:, gc, stt * 128:(stt + 1) * 128],
                                         wc_sb[:, gc, ee * 512:(ee + 1) * 512],
                                         start=(gi == 0), stop=(gi == 1),
                                         skip_group_check=True)
                ost = ost_p.tile([128, E], BF16, tag="ost", name=f"ost{part}_{stt}")
                if on_act:
                    nc.scalar.activation(ost[:, :], pc[:, :], ACT_COPY)
                else:
                    nc.vector.tensor_copy(ost[:, :], pc[:, :])
                nc.sync.dma_start(out=outp[part, stt * 128:(stt + 1) * 128, :],
                                  in_=ost[:, :])

            # ---------- phase-2 per-head attention ----------
            state = {"norm": None, "diag": None, "pv": None, "bcst": None}

            def normalize_a(h, po):
                """reciprocal of the denominator row + GPSIMD partition
                broadcast, chunked so Pool-queue neighbors (affine_select)
                are not delayed much"""
                rcp = rcp_p.tile([1, S], F32R, tag="rcp", name=f"rcp{h}")
                bcst = bcst_p.tile([64, S], F32R, tag="bcst", name=f"bcst{h}")
                nc.vector.reciprocal(rcp, po[64:65, :])
                for grp in range(4):
                    gs = slice(grp * 512, (grp + 1) * 512)
                    nc.gpsimd.partition_broadcast(bcst[:, gs], rcp[0:1, gs])
                return bcst

            def normalize_b(h, po, bcst):
                """oT[d-half, pair, :] = po[0:64, :] * (1/den) (bf16)"""
                m, half = h // 2, h % 2
                for grp in range(4):
                    gs = slice(grp * 512, (grp + 1) * 512)
                    nc.vector.tensor_mul(
                        oT[64 * half:64 * half + 64, m, gs],
                        po[0:64, gs], bcst[:, gs])

            def head_attention(h, fillers):
                """fillers: dict kc -> list of thunks emitted at that k-chunk."""
                m, half = h // 2, h % 2
                qt = qkt[m][64 * half:64 * half + 64, :]
                kt = qkt[4 + m][64 * half:64 * half + 64, :]
                po = None
                pocell = {}
                for kc in range(NT):
                    q0 = 128 * kc
                    ptile = pt_p.tile([128, 2048], BF16, tag="pt",
                                      name=f"pt{h}_{kc}")
                    for c0 in range(q0, S, 1024):
                        cw = min(1024, S - c0)
                        st = stp_p.tile([128, 1024], F32, tag="stp",
                                        name=f"st{h}_{kc}_{c0}")
                        for u0 in range(c0, c0 + cw, 512):
                            uw = min(512, c0 + cw - u0)
                            nc.tensor.matmul(st[:, u0 - c0:u0 - c0 + uw],
                                             kt[:, q0:q0 + 128], qt[:, u0:u0 + uw],
                                             start=True, stop=True)
                        nc.scalar.activation(ptile[:, c0 - q0:c0 - q0 + cw],
                                             st[:, 0:cw], ACT_EXP, scale=0.125)
                    dg = diag_p.tile([128, 128], BF16, tag="dg",
                                     name=f"dg{h}_{kc}")
                    nc.gpsimd.affine_select(
                        out=dg, in_=ptile[:, 0:128],
                        compare_op=mybir.AluOpType.is_ge, fill=0.0,
                        base=0, pattern=[[1, 128]], channel_multiplier=-1)
                    if kc == 0 and state["norm"] is not None:
                        # previous head's normalization, stage a (rcp +
                        # Pool broadcasts run under this head's first scores)
                        state["bcst"] = normalize_a(*state["norm"])
                    if kc == 1:
                        if state["norm"] is not None:
                            normalize_b(*state["norm"], state["bcst"])
                            state["norm"] = None
                        # po slot reuse is safe only after the deferred
                        # normalization above has been emitted
                        po = po_p.tile([128, S], F32, tag="po", name=f"po{h}")
                        pocell["po"] = po
                        # zero bank 0 so every PV there can accumulate with
                        # start=False (PSUM pending-zero rules)
                        nc.tensor.matmul(po[0:65, 0:512], zrow[0:1, 0:65],
                                         zrow[0:1, 0:512],
                                         start=True, stop=False,
                                         skip_group_check=True)
                    # PV runs one k-chunk behind the scores so the PE never
                    # waits on the exp it just requested: PV(kc-1) reads a
                    # ptile whose exp finished during scores(kc)
                    if state["pv"] is not None:
                        state["pv"]()
                        state["pv"] = None
                    if state["diag"] is not None:
                        state["diag"]()
                        state["diag"] = None

                    def pv_pieces(kcc=kc, ptt=ptile, hh=h):
                        poo = pocell["po"]
                        # non-diagonal PV: q in [q0+128, S), 512-aligned
                        # pieces; kc==0 pieces outside bank 0 open their
                        # accumulation group (bank 0 was pre-zeroed)
                        a = 128 * kcc + 128
                        while a < S:
                            b_ = min(S, (a // 512 + 1) * 512)
                            nc.tensor.matmul(poo[0:65, a:b_],
                                             v65[:, kcc, hh, 0:65],
                                             ptt[:, a - 128 * kcc:b_ - 128 * kcc],
                                             start=(kcc == 0 and a >= 512),
                                             stop=False, skip_group_check=True)
                            a = b_

                    state["pv"] = pv_pieces
                    state["diag"] = (
                        lambda kcc=kc, dgg=dg, hh=h: nc.tensor.matmul(
                            pocell["po"][0:65, kcc * 128:(kcc + 1) * 128],
                            v65[:, kcc, hh, 0:65], dgg[:, :],
                            start=False, stop=True, skip_group_check=True))
                    for th in fillers.get(kc, ()):
                        th()
                state["pv"]()
                state["pv"] = None
                state["diag"]()
                state["diag"] = None
                state["norm"] = (h, pocell["po"])

            # ---------- schedule ----------
            # warm the PE clock (p-state ramps over ~3us of busy time)
            # while the input DMAs land
            for w in range(8):
                wt = stp_p.tile([128, 1024], F32, tag="stp", name=f"warm{w}")
                nc.tensor.matmul(wt[:, 0:512], zrow[0:1, 0:128],
                                 zrow[0:1, 0:512], start=True, stop=True,
                                 skip_group_check=True)
            # prefix: head 0's q must be complete (scores read qt[:, q0:S]);
            # its k and v arrive narrowly (kt[:, q0:q0+128] / v65[:, kc]) so
            # later chunks stream in as filler
            for m_, ss_ in ((0, 0), (0, 1), (4, 0), (0, 2), (0, 3)):
                qk_chain(m_, ss_)
            v_chain(0)

            def QK(m, ss):
                return lambda: qk_chain(m, ss)

            def VC(*stts):
                return lambda: [v_chain(s) for s in stts]

            def CP(gc, *stts):
                return lambda: [cproj_chunk(gc, s) for s in stts]

            # filler placement rules: a chain emitted at slot kc is only
            # readable from chunk kc+1 on (reads emitted before writes see
            # stale data).  v65[:, k] is needed at PV(k); kt chunk ss at
            # scores(4*ss); q chunks must be complete before the head starts.
            fillers = {
                0: {0: [VC(1)], 1: [QK(4, 1), VC(2)], 2: [VC(3, 4)],
                    3: [VC(5, 6)], 5: [QK(4, 2), VC(7, 8)], 7: [VC(9, 10)],
                    9: [QK(4, 3), VC(11, 12)], 11: [VC(13, 14)], 13: [VC(15)]},
                1: {1: [QK(1, 0)], 3: [QK(5, 0)], 5: [QK(1, 1)], 7: [QK(5, 1)],
                    9: [QK(1, 2)], 11: [QK(5, 2)], 13: [QK(1, 3), QK(5, 3)]},
                2: {1: [QK(2, 0)], 3: [QK(6, 0)], 5: [QK(2, 1)], 7: [QK(6, 1)]},
                3: {1: [QK(2, 2)], 3: [QK(6, 2)], 5: [QK(2, 3)], 7: [QK(6, 3)]},
                4: {1: [QK(3, 0)], 3: [QK(7, 0)], 5: [QK(3, 1)], 7: [QK(7, 1)],
                    9: [CP(0, 0)], 11: [CP(0, 1)], 13: [CP(0, 2)]},
                5: {1: [QK(3, 2)], 3: [QK(7, 2)], 5: [QK(3, 3)], 7: [QK(7, 3)],
                    9: [CP(0, 3)], 11: [CP(0, 4)], 13: [CP(0, 5)]},
                6: {1: [CP(0, 6)], 3: [CP(0, 7)], 5: [CP(0, 8)], 7: [CP(0, 9)],
                    9: [CP(0, 10)], 11: [CP(0, 11)], 13: [CP(0, 12)]},
                7: {1: [CP(0, 13)], 3: [CP(0, 14)], 5: [CP(0, 15)]},
            }
            for h in range(8):
                head_attention(h, fillers[h])
            normalize_b(*state["norm"], normalize_a(*state["norm"]))
            # tail: second half of c_proj, PSUM evacuation split between DVE
            # and the now-idle Act engine
            for stt in range(NT):
                cproj_chunk(1, stt, on_act=(stt % 2 == 1))

    nc.compile()
    return nc


def _fp8(a, scale=1.0):
    return (np.asarray(a, np.float32) * scale).astype(ml_dtypes.float8_e4m3)


def _bf16(a):
    return np.asarray(a, np.float32).astype(ml_dtypes.bfloat16)


def prep_core_inputs(hidden_states, position_states, Wq, bq, Wqh, bqh, Wk, bk,
                     Wkh, bkh, Wv, bv, Wvh, bvh, Wp, bp, Wpe, bpe, Wc, bc):
    """Host-side weight folding + per-core staging."""
    f32 = np.float32
    eyeE = np.eye(E, dtype=f32)

    def fold(parity):
        hs = slice(G * parity, G * parity + G)
        csl = slice(512 * parity, 512 * parity + 512)
        mats = {}
        for name, (Wa, ba, Wh, bh, v) in {
            "q": (Wq, bq, Wqh[hs], bqh[hs], 0),
            "k": (Wk, bk, Wkh[hs], bkh[hs], 1),
            "v": (Wv, bv, Wvh[hs], bvh[hs], 2),
        }.items():
            mx = np.einsum("hed,ghd->hegd", Wa, Wh).reshape(E, 512)
            mp = np.einsum("pd,g->pgd", Wp[:, v * D:(v + 1) * D],
                           Wpe[v, 0, hs]).reshape(P, 512)
            bias = (np.einsum("hd,ghd->gd", ba, Wh) + bh
                    + bp[v * D:(v + 1) * D][None, :] * Wpe[v, 0, hs][:, None]
                    + bpe[hs][:, None]).reshape(512)
            C = np.zeros((NPAIR * 256, 512), f32)
            C[:E] = mx - eyeE[:, csl]
            C[E:E + P] = mp
            C[E + P] = bias
            mats[name] = C
        cqk = np.concatenate([mats["q"], mats["k"]], axis=1)     # [1280, 1024]
        cqk8 = _fp8(cqk.reshape(NPAIR, 2, 128, 1024).transpose(2, 0, 1, 3), CSCALE)
        cv8 = _fp8(mats["v"].reshape(NPAIR, 2, 128, 512).transpose(2, 0, 1, 3), CSCALE)
        wc = Wc.reshape(H, D, E)[hs].reshape(512, E).reshape(4, 128, E).transpose(1, 0, 2)
        return (np.ascontiguousarray(cqk8), np.ascontiguousarray(cv8),
                np.ascontiguousarray(_bf16(wc)))

    per_parity = [fold(0), fold(1)]

    in_maps = []
    for c in range(NCORE):
        b, parity = c // 2, c % 2
        csl = slice(512 * parity, 512 * parity + 512)
        xaug = np.zeros((NPAIR * 256, S), f32)
        xaug[:E] = hidden_states[b].T
        xaug[E:E + P] = position_states[b].T
        xaug[E + P] = 1.0
        x8 = _fp8(xaug)
        dx = xaug[:E] - x8[:E].astype(f32)
        xt8 = np.ascontiguousarray(
            x8.reshape(NPAIR, 2, 128, S).transpose(2, 0, 1, 3))
        dxt8 = np.ascontiguousarray(
            _fp8(dx).reshape(4, 2, 128, S).transpose(2, 0, 1, 3))
        xtid = np.ascontiguousarray(
            _bf16(hidden_states[b].T[csl]).reshape(4, 128, S).transpose(1, 0, 2))
        xnat = np.ascontiguousarray(
            _bf16(hidden_states[b][:, csl]).reshape(NT, 128, 512).transpose(1, 0, 2))
        cqk8, cv8, wc = per_parity[parity]
        in_maps.append({"cqk8": cqk8, "cv8": cv8, "xt8": xt8, "dxt8": dxt8,
                        "xtid": xtid, "xnat": xnat, "wc16": wc,
                        "onesb": _bf16(np.ones((128, 128), f32)),
                        "zerob": _bf16(np.zeros((1, 512), f32))})
    return in_maps


_NC_CACHE = {}


def get_nc():
    if "nc" not in _NC_CACHE:
        _NC_CACHE["nc"] = build_nc()
    return _NC_CACHE["nc"]


def assemble(results, bc):
    outs = []
    for b in range(B):
        acc = np.zeros((S, E), np.float32)
        for p in range(2):
            acc += results[2 * b + p]["outp"].astype(np.float32).sum(axis=0)
        outs.append(acc + bc)
    return np.stack(outs).astype(np.float32)


def kernel(**inputs):
    nc = get_nc()
    in_maps = prep_core_inputs(**inputs)
    res = run_bass_kernel_spmd(nc, in_maps, list(range(NCORE)))
    return assemble(res.results, inputs["bc"])


# revision 19
# speedup vs baseline: 1.2798x; 1.0321x over previous
"""Trainium2 Bass kernel for nn_Attention_77025943487081.

Sharding: batch (4) data-parallel x 2-way head tensor-parallel over 8 cores.
Core c handles batch c//2 and heads [8*(c%2), 8*(c%2)+8). Each core emits 4
partial c_proj outputs (one per head-pair, bf16); the host sums the 8 partials
per batch and adds the c_proj bias.

Numerics (validated in numpy against the f32 reference, rel err ~8e-3 vs the
2e-2 gate):
  - The folded per-head+cross-head projection matrices are I + C with C at
    0.02 scale.  q/k/v = bf16(x_slice) + (x8 + dx8) @ C8 / 32 where x8/dx8
    are fp8e4m3 value+residual and C8 = fp8(32*C) (the 32x pre-scale keeps
    C's entries out of fp8's subnormal range). The correction matmuls run in
    fp8 DoubleRow mode (256-deep contraction, 0.5 cycles/row).
  - Scores/PV/c_proj operands are bf16, accumulation always f32 PSUM.
  - The 1/sqrt(D) score scale is applied inside the softmax exp activation
    (out = exp(in * 0.125)), so q is staged unscaled.

Attention per head: scores^T [k, q] per 128-wide k-chunk -> exp on Act ->
PV in [d, q] orientation (po65[65, S] accumulator; row 64 collects softmax
denominators via a ones row appended to V). Wide PV matmuls (<=512 q
columns) keep the PE instruction count low — per-instruction semaphore and
issue overhead (~50-100 ns) is what kills many-small-matmul schedules. The
diagonal (causally masked) block's PV is deferred one k-chunk so the Pool
affine_select round-trip hides behind the next chunk's scores.

Schedule: the Act engine paces each head (~19 us of softmax exp vs ~16 us
of PE work), so the remaining phase-1 chains and the previous pair's
partial c_proj run as filler inside later heads' k-chunk loops. Each head's
normalization (1/den broadcast via a ones-column matmul) is deferred into
the next head's first k-chunk. The final pair's c_proj evacuates PSUM on
both DVE and Act (Act is idle by then).
"""

import numpy as np
import ml_dtypes
from contextlib import ExitStack

import concourse.bass as bass
import concourse.tile as tile
from concourse import bacc, mybir
from concourse.bass_utils import run_bass_kernel_spmd

F32 = mybir.dt.float32
F32R = mybir.dt.float32r
BF16 = mybir.dt.bfloat16
FP8 = mybir.dt.float8e4
DR = mybir.MatmulPerfMode.DoubleRow
ACT_EXP = mybir.ActivationFunctionType.Exp
ACT_COPY = mybir.ActivationFunctionType.Copy
MULT = mybir.AluOpType.mult
ADD = mybir.AluOpType.add

B, S, E, H, D, P = 4, 2048, 1024, 16, 64, 64
G = 8            # heads per core
NCORE = 8
NT = S // 128    # 16 sequence tiles
NPAIR = 5        # fp8 DoubleRow contraction pairs: 1280 = 5*256 rows (1089 used)
CSCALE = 32.0    # fp8 pre-scale on the correction matrices
INV_CS = 1.0 / CSCALE


def build_nc():
    nc = bacc.Bacc("TRN2", target_bir_lowering=False, debug=False, num_devices=NCORE)
    cqk8 = nc.dram_tensor("cqk8", [128, NPAIR, 2, 1024], FP8, kind="ExternalInput").ap()
    cv8 = nc.dram_tensor("cv8", [128, NPAIR, 2, 512], FP8, kind="ExternalInput").ap()
    xt8 = nc.dram_tensor("xt8", [128, NPAIR, 2, S], FP8, kind="ExternalInput").ap()
    dxt8 = nc.dram_tensor("dxt8", [128, 4, 2, S], FP8, kind="ExternalInput").ap()
    xtid = nc.dram_tensor("xtid", [128, 4, S], BF16, kind="ExternalInput").ap()
    xnat = nc.dram_tensor("xnat", [128, NT, 512], BF16, kind="ExternalInput").ap()
    wc16 = nc.dram_tensor("wc16", [128, 4, E], BF16, kind="ExternalInput").ap()
    onesb = nc.dram_tensor("onesb", [128, 128], BF16, kind="ExternalInput").ap()
    zerob = nc.dram_tensor("zerob", [1, 512], BF16, kind="ExternalInput").ap()
    outp = nc.dram_tensor("outp", [2, S, E], BF16, kind="ExternalOutput").ap()

    with nc.allow_low_precision("bf16/fp8 staged operands; f32 PSUM accumulation"), \
         tile.TileContext(nc) as tc, ExitStack() as top:
        const_p = top.enter_context(tc.tile_pool(name="const", bufs=1))
        qk_p = top.enter_context(tc.tile_pool(name="qkt", bufs=1))
        vaug_p = top.enter_context(tc.tile_pool(name="vaug", bufs=1))
        oT_p = top.enter_context(tc.tile_pool(name="oT", bufs=1))

        cqk_sb = const_p.tile([128, NPAIR, 2, 1024], FP8)
        cv_sb = const_p.tile([128, NPAIR, 2, 512], FP8)
        xt_sb = const_p.tile([128, NPAIR, 2, S], FP8)
        dxt_sb = const_p.tile([128, 4, 2, S], FP8)
        xtid_sb = const_p.tile([128, 4, S], BF16)
        xnat_sb = const_p.tile([128, NT, 512], BF16)
        wc_sb = const_p.tile([128, 4, E], BF16)
        zrow = const_p.tile([1, 512], BF16)

        qkt = [qk_p.tile([128, S], BF16, name=f"qkt{m}") for m in range(8)]
        v65 = vaug_p.tile([128, NT, G, 66], BF16)   # col 64 = ones (denominator)
        oT = oT_p.tile([128, 4, S], BF16)

        # --- DMAs, startup-critical first -------------------------------
        # each DMA pays ~625ns serialized HWDGE overhead + ~900ns sem
        # propagation, so transfers are consolidated (DRAM layouts mirror the
        # SBUF tiles, partition-major)
        nc.sync.dma_start(out=zrow, in_=zerob)
        for col in (64, 65):
            nc.sync.dma_start(
                out=v65[:, :, :, col:col + 1],
                in_=onesb[:, 0:128].rearrange("p (a b c) -> p a b c", a=16, b=8, c=1))
        # stage 1: what the first chains (m0/m4, all pairs) + head-0 kc0 need
        nc.sync.dma_start(out=cqk_sb[:, :, :, 0:128], in_=cqk8[:, :, :, 0:128])
        nc.sync.dma_start(out=cqk_sb[:, :, :, 512:640], in_=cqk8[:, :, :, 512:640])
        nc.sync.dma_start(out=xt_sb[:, :, :, 0:1024], in_=xt8[:, :, :, 0:1024])
        nc.sync.dma_start(out=dxt_sb[:, :, :, 0:1024], in_=dxt8[:, :, :, 0:1024])
        nc.sync.dma_start(out=xtid_sb[:, 0, 0:1024], in_=xtid[:, 0, 0:1024])
        # stage 2: second halves, v-path, remaining weights
        nc.sync.dma_start(out=xt_sb[:, :, :, 1024:2048], in_=xt8[:, :, :, 1024:2048])
        nc.sync.dma_start(out=dxt_sb[:, :, :, 1024:2048],
                          in_=dxt8[:, :, :, 1024:2048])
        nc.sync.dma_start(out=xtid_sb[:, 0, 1024:2048], in_=xtid[:, 0, 1024:2048])
        nc.sync.dma_start(out=cv_sb, in_=cv8)
        nc.sync.dma_start(out=xnat_sb[:, 0:8], in_=xnat[:, 0:8])
        nc.sync.dma_start(out=xnat_sb[:, 8:NT], in_=xnat[:, 8:NT])
        nc.sync.dma_start(out=cqk_sb[:, :, :, 128:512], in_=cqk8[:, :, :, 128:512])
        nc.sync.dma_start(out=cqk_sb[:, :, :, 640:1024],
                          in_=cqk8[:, :, :, 640:1024])
        nc.sync.dma_start(out=xtid_sb[:, 1:4], in_=xtid[:, 1:4])
        nc.sync.dma_start(out=wc_sb, in_=wc16)

        with tc.tile_pool(name="stp", bufs=2, space="PSUM") as stp_p, \
             tc.tile_pool(name="pop", bufs=1, space="PSUM") as po_p, \
             tc.tile_pool(name="pt", bufs=4) as pt_p, \
             tc.tile_pool(name="diag", bufs=2) as diag_p, \
             tc.tile_pool(name="bcst", bufs=1) as bcst_p, \
             tc.tile_pool(name="rcpp", bufs=1) as rcp_p, \
             tc.tile_pool(name="ost", bufs=3) as ost_p:

            # ---------- phase-1 building blocks ----------
            def qk_chain(m, ss):
                """qkt[m][:, ss*512:+512] = identity x-slice + fp8 correction."""
                ps = stp_p.tile([128, 1024], F32, tag="stp", name=f"qkps{m}_{ss}")
                side = 0 if m < 4 else 512
                col0 = side + (m % 4) * 128
                xsl = slice(ss * 512, (ss + 1) * 512)
                for pr in range(NPAIR):
                    nc.tensor.matmul(ps[:, 0:512],
                                     cqk_sb[:, pr, :, col0:col0 + 128],
                                     xt_sb[:, pr, :, xsl],
                                     start=(pr == 0), stop=False, perf_mode=DR)
                for pr in range(4):
                    nc.tensor.matmul(ps[:, 0:512],
                                     cqk_sb[:, pr, :, col0:col0 + 128],
                                     dxt_sb[:, pr, :, xsl],
                                     start=False, stop=(pr == 3), perf_mode=DR)
                nc.vector.scalar_tensor_tensor(
                    out=qkt[m][:, xsl], in0=ps[:, 0:512], scalar=INV_CS,
                    in1=xtid_sb[:, m % 4, xsl], op0=MULT, op1=ADD)

            def v_chain(stt):
                pv = stp_p.tile([128, 1024], F32, tag="stp", name=f"vps{stt}")
                for pr in range(NPAIR):
                    nc.tensor.matmul(pv[:, 0:512],
                                     xt_sb[:, pr, :, stt * 128:(stt + 1) * 128],
                                     cv_sb[:, pr, :, :],
                                     start=(pr == 0), stop=(pr == NPAIR - 1),
                                     perf_mode=DR)
                nc.vector.scalar_tensor_tensor(
                    out=v65[:, stt, :, 0:64],
                    in0=pv[:, 0:512].rearrange("p (g d) -> p g d", g=G),
                    scalar=INV_CS,
                    in1=xnat_sb[:, stt].rearrange("p (g d) -> p g d", g=G),
                    op0=MULT, op1=ADD)

            def cproj_chunk(part, stt, on_act=False):
                """one 128-row slab of the half c_proj (head-pairs 2p, 2p+1)"""
                pc = stp_p.tile([128, 1024], F32, tag="stp", name=f"pc{part}_{stt}")
                for gi, gc in enumerate((2 * part, 2 * part + 1)):
                    for ee in range(2):
                        nc.tensor.matmul(pc[:, ee * 512:(ee + 1) * 512],
                                         oT[:, gc, stt * 128:(stt + 1) * 128],
                                         wc_sb[:, gc, ee * 512:(ee + 1) * 512],
                                         start=(gi == 0), stop=(gi == 1),
                                         skip_group_check=True)
                ost = ost_p.tile([128, E], BF16, tag="ost", name=f"ost{part}_{stt}")
                if on_act:
                    nc.scalar.activation(ost[:, :], pc[:, :], ACT_COPY)
                else:
                    nc.vector.tensor_copy(ost[:, :], pc[:, :])
                nc.sync.dma_start(out=outp[part, stt * 128:(stt + 1) * 128, :],
                                  in_=ost[:, :])

            # ---------- phase-2 per-head attention ----------
            state = {"norm": None, "diag": None, "pv": None, "bcst": None}

            def normalize_a(h, po):
                """reciprocal of the denominator row + GPSIMD partition
                broadcast, chunked so Pool-queue neighbors (affine_select)
                are not delayed much"""
                rcp = rcp_p.tile([1, S], F32R, tag="rcp", name=f"rcp{h}")
                bcst = bcst_p.tile([64, S], F32R, tag="bcst", name=f"bcst{h}")
                nc.vector.reciprocal(rcp, po[64:65, :])
                for grp in range(4):
                    gs = slice(grp * 512, (grp + 1) * 512)
                    nc.gpsimd.partition_broadcast(bcst[:, gs], rcp[0:1, gs])
                return bcst

            def normalize_b(h, po, bcst):
                """oT[d-half, pair, :] = po[0:64, :] * (1/den) (bf16)"""
                m, half = h // 2, h % 2
                for grp in range(4):
                    gs = slice(grp * 512, (grp + 1) * 512)
                    nc.vector.tensor_mul(
                        oT[64 * half:64 * half + 64, m, gs],
                        po[0:64, gs], bcst[:, gs])

            def head_attention(h, fillers):
                """fillers: dict kc -> list of thunks emitted at that k-chunk."""
                m, half = h // 2, h % 2
                qt = qkt[m][64 * half:64 * half + 64, :]
                kt = qkt[4 + m][64 * half:64 * half + 64, :]
                po = None
                pocell = {}
                for kc in range(NT):
                    q0 = 128 * kc
                    ptile = pt_p.tile([128, 2048], BF16, tag="pt",
                                      name=f"pt{h}_{kc}")
                    for c0 in range(q0, S, 1024):
                        cw = min(1024, S - c0)
                        st = stp_p.tile([128, 1024], F32, tag="stp",
                                        name=f"st{h}_{kc}_{c0}")
                        for u0 in range(c0, c0 + cw, 512):
                            uw = min(512, c0 + cw - u0)
                            nc.tensor.matmul(st[:, u0 - c0:u0 - c0 + uw],
                                             kt[:, q0:q0 + 128], qt[:, u0:u0 + uw],
                                             start=True, stop=True)
                        nc.scalar.activation(ptile[:, c0 - q0:c0 - q0 + cw],
                                             st[:, 0:cw], ACT_EXP, scale=0.125)
                    dg = diag_p.tile([128, 128], BF16, tag="dg",
                                     name=f"dg{h}_{kc}")
                    nc.gpsimd.affine_select(
                        out=dg, in_=ptile[:, 0:128],
                        compare_op=mybir.AluOpType.is_ge, fill=0.0,
                        base=0, pattern=[[1, 128]], channel_multiplier=-1)
                    if kc == 0 and state["norm"] is not None:
                        # previous head's normalization, stage a (rcp +
                        # Pool broadcasts run under this head's first scores)
                        state["bcst"] = normalize_a(*state["norm"])
                    if kc == 1:
                        if state["norm"] is not None:
                            normalize_b(*state["norm"], state["bcst"])
                            state["norm"] = None
                        # po slot reuse is safe only after the deferred
                        # normalization above has been emitted
                        po = po_p.tile([128, S], F32, tag="po", name=f"po{h}")
                        pocell["po"] = po
                        # zero bank 0 so every PV there can accumulate with
                        # start=False (PSUM pending-zero rules)
                        nc.tensor.matmul(po[0:65, 0:512], zrow[0:1, 0:65],
                                         zrow[0:1, 0:512],
                                         start=True, stop=False,
                                         skip_group_check=True)
                    # PV runs one k-chunk behind the scores so the PE never
                    # waits on the exp it just requested: PV(kc-1) reads a
                    # ptile whose exp finished during scores(kc)
                    if state["pv"] is not None:
                        state["pv"]()
                        state["pv"] = None
                    if state["diag"] is not None:
                        state["diag"]()
                        state["diag"] = None

                    def pv_pieces(kcc=kc, ptt=ptile, hh=h):
                        poo = pocell["po"]
                        # non-diagonal PV: q in [q0+128, S), 512-aligned
                        # pieces; kc==0 pieces outside bank 0 open their
                        # accumulation group (bank 0 was pre-zeroed)
                        a = 128 * kcc + 128
                        while a < S:
                            b_ = min(S, (a // 512 + 1) * 512)
                            nc.tensor.matmul(poo[0:65, a:b_],
                                             v65[:, kcc, hh, 0:65],
                                             ptt[:, a - 128 * kcc:b_ - 128 * kcc],
                                             start=(kcc == 0 and a >= 512),
                                             stop=False, skip_group_check=True)
                            a = b_

                    state["pv"] = pv_pieces
                    state["diag"] = (
                        lambda kcc=kc, dgg=dg, hh=h: nc.tensor.matmul(
                            pocell["po"][0:65, kcc * 128:(kcc + 1) * 128],
                            v65[:, kcc, hh, 0:65], dgg[:, :],
                            start=False, stop=True, skip_group_check=True))
                    for th in fillers.get(kc, ()):
                        th()
                state["pv"]()
                state["pv"] = None
                state["diag"]()
                state["diag"] = None
                state["norm"] = (h, pocell["po"])

            # ---------- schedule ----------
            # warm the PE clock (p-state ramps over ~3us of busy time)
            # while the input DMAs land
            for w in range(8):
                wt = stp_p.tile([128, 1024], F32, tag="stp", name=f"warm{w}")
                nc.tensor.matmul(wt[:, 0:512], zrow[0:1, 0:128],
                                 zrow[0:1, 0:512], start=True, stop=True,
                                 skip_group_check=True)
            # prefix: head 0's q must be complete (scores read qt[:, q0:S]);
            # its k and v arrive narrowly (kt[:, q0:q0+128] / v65[:, kc]) so
            # later chunks stream in as filler
            for m_, ss_ in ((0, 0), (0, 1), (4, 0), (0, 2), (0, 3)):
                qk_chain(m_, ss_)
            v_chain(0)

            def QK(m, ss):
                return lambda: qk_chain(m, ss)

            def VC(*stts):
                return lambda: [v_chain(s) for s in stts]

            def CP(gc, *stts):
                return lambda: [cproj_chunk(gc, s) for s in stts]

            # filler placement rules: a chain emitted at slot kc is only
            # readable from chunk kc+1 on (reads emitted before writes see
            # stale data).  v65[:, k] is needed at PV(k); kt chunk ss at
            # scores(4*ss); q chunks must be complete before the head starts.
            fillers = {
                0: {0: [VC(1)], 1: [QK(4, 1), VC(2)], 2: [VC(3, 4)],
                    3: [VC(5, 6)], 5: [QK(4, 2), VC(7, 8)], 7: [VC(9, 10)],
                    9: [QK(4, 3), VC(11, 12)], 11: [VC(13, 14)], 13: [VC(15)]},
                1: {1: [QK(1, 0)], 3: [QK(5, 0)], 5: [QK(1, 1)], 7: [QK(5, 1)],
                    9: [QK(1, 2)], 11: [QK(5, 2)], 13: [QK(1, 3), QK(5, 3)]},
                2: {1: [QK(2, 0)], 3: [QK(6, 0)], 5: [QK(2, 1)], 7: [QK(6, 1)]},
                3: {1: [QK(2, 2)], 3: [QK(6, 2)], 5: [QK(2, 3)], 7: [QK(6, 3)]},
                4: {1: [QK(3, 0)], 3: [QK(7, 0)], 5: [QK(3, 1)], 7: [QK(7, 1)],
                    9: [CP(0, 0)], 11: [CP(0, 1)], 13: [CP(0, 2)]},
                5: {1: [QK(3, 2)], 3: [QK(7, 2)], 5: [QK(3, 3)], 7: [QK(7, 3)],
                    9: [CP(0, 3)], 11: [CP(0, 4)], 13: [CP(0, 5)]},
                6: {1: [CP(0, 6)], 3: [CP(0, 7)], 5: [CP(0, 8)], 7: [CP(0, 9)],
                    9: [CP(0, 10)], 11: [CP(0, 11)], 13: [CP(0, 12)]},
                7: {1: [CP(0, 13)], 3: [CP(0, 14)], 5: [CP(0, 15)]},
            }
            for h in range(8):
                head_attention(h, fillers[h])
            normalize_b(*state["norm"], normalize_a(*state["norm"]))
            # tail: second half of c_proj, PSUM evacuation split between DVE
            # and the now-idle Act engine
            for stt in range(NT):
                cproj_chunk(1, stt, on_act=(stt % 2 == 1))

    nc.compile()
    return nc


def _fp8(a, scale=1.0):
    return (np.asarray(a, np.float32) * scale).astype(ml_dtypes.float8_e4m3)


def _bf16(a):
    return np.asarray(a, np.float32).astype(ml_dtypes.bfloat16)


def prep_core_inputs(hidden_states, position_states, Wq, bq, Wqh, bqh, Wk, bk,
                     Wkh, bkh, Wv, bv, Wvh, bvh, Wp, bp, Wpe, bpe, Wc, bc):
    """Host-side weight folding + per-core staging."""
    f32 = np.float32
    eyeE = np.eye(E, dtype=f32)

    def fold(parity):
        hs = slice(G * parity, G * parity + G)
        csl = slice(512 * parity, 512 * parity + 512)
        mats = {}
        for name, (Wa, ba, Wh, bh, v) in {
            "q": (Wq, bq, Wqh[hs], bqh[hs], 0),
            "k": (Wk, bk, Wkh[hs], bkh[hs], 1),
            "v": (Wv, bv, Wvh[hs], bvh[hs], 2),
        }.items():
            mx = np.einsum("hed,ghd->hegd", Wa, Wh).reshape(E, 512)
            mp = np.einsum("pd,g->pgd", Wp[:, v * D:(v + 1) * D],
                           Wpe[v, 0, hs]).reshape(P, 512)
            bias = (np.einsum("hd,ghd->gd", ba, Wh) + bh
                    + bp[v * D:(v + 1) * D][None, :] * Wpe[v, 0, hs][:, None]
                    + bpe[hs][:, None]).reshape(512)
            C = np.zeros((NPAIR * 256, 512), f32)
            C[:E] = mx - eyeE[:, csl]
            C[E:E + P] = mp
            C[E + P] = bias
            mats[name] = C
        cqk = np.concatenate([mats["q"], mats["k"]], axis=1)     # [1280, 1024]
        cqk8 = _fp8(cqk.reshape(NPAIR, 2, 128, 1024).transpose(2, 0, 1, 3), CSCALE)
        cv8 = _fp8(mats["v"].reshape(NPAIR, 2, 128, 512).transpose(2, 0, 1, 3), CSCALE)
        wc = Wc.reshape(H, D, E)[hs].reshape(512, E).reshape(4, 128, E).transpose(1, 0, 2)
        return (np.ascontiguousarray(cqk8), np.ascontiguousarray(cv8),
                np.ascontiguousarray(_bf16(wc)))

    per_parity = [fold(0), fold(1)]

    in_maps = []
    for c in range(NCORE):
        b, parity = c // 2, c % 2
        csl = slice(512 * parity, 512 * parity + 512)
        xaug = np.zeros((NPAIR * 256, S), f32)
        xaug[:E] = hidden_states[b].T
        xaug[E:E + P] = position_states[b].T
        xaug[E + P] = 1.0
        x8 = _fp8(xaug)
        dx = xaug[:E] - x8[:E].astype(f32)
        xt8 = np.ascontiguousarray(
            x8.reshape(NPAIR, 2, 128, S).transpose(2, 0, 1, 3))
        dxt8 = np.ascontiguousarray(
            _fp8(dx).reshape(4, 2, 128, S).transpose(2, 0, 1, 3))
        xtid = np.ascontiguousarray(
            _bf16(hidden_states[b].T[csl]).reshape(4, 128, S).transpose(1, 0, 2))
        xnat = np.ascontiguousarray(
            _bf16(hidden_states[b][:, csl]).reshape(NT, 128, 512).transpose(1, 0, 2))
        cqk8, cv8, wc = per_parity[parity]
        in_maps.append({"cqk8": cqk8, "cv8": cv8, "xt8": xt8, "dxt8": dxt8,
                        "xtid": xtid, "xnat": xnat, "wc16": wc,
                        "onesb": _bf16(np.ones((128, 128), f32)),
                        "zerob": _bf16(np.zeros((1, 512), f32))})
    return in_maps


_NC_CACHE = {}


def get_nc():
    if "nc" not in _NC_CACHE:
        _NC_CACHE["nc"] = build_nc()
    return _NC_CACHE["nc"]


def assemble(results, bc):
    outs = []
    for b in range(B):
        acc = np.zeros((S, E), np.float32)
        for p in range(2):
            acc += results[2 * b + p]["outp"].astype(np.float32).sum(axis=0)
        outs.append(acc + bc)
    return np.stack(outs).astype(np.float32)


def kernel(**inputs):
    nc = get_nc()
    in_maps = prep_core_inputs(**inputs)
    res = run_bass_kernel_spmd(nc, in_maps, list(range(NCORE)))
    return assemble(res.results, inputs["bc"])


# revision 20
# speedup vs baseline: 1.3412x; 1.0480x over previous
"""Trainium2 Bass kernel for nn_Attention_77025943487081.

Sharding: batch (4) data-parallel x 2-way head tensor-parallel over 8 cores.
Core c handles batch c//2 and heads [8*(c%2), 8*(c%2)+8). Each core emits 4
partial c_proj outputs (one per head-pair, bf16); the host sums the 8 partials
per batch and adds the c_proj bias.

Numerics (validated in numpy against the f32 reference, rel err ~8e-3 vs the
2e-2 gate):
  - The folded per-head+cross-head projection matrices are I + C with C at
    0.02 scale.  q/k/v = bf16(x_slice) + (x8 + dx8) @ C8 / 32 where x8/dx8
    are fp8e4m3 value+residual and C8 = fp8(32*C) (the 32x pre-scale keeps
    C's entries out of fp8's subnormal range). The correction matmuls run in
    fp8 DoubleRow mode (256-deep contraction, 0.5 cycles/row).
  - Scores/PV/c_proj operands are bf16, accumulation always f32 PSUM.
  - The 1/sqrt(D) score scale is applied inside the softmax exp activation
    (out = exp(in * 0.125)), so q is staged unscaled.

Attention per head: scores^T [k, q] per 128-wide k-chunk -> exp on Act ->
PV in [d, q] orientation (po65[65, S] accumulator; row 64 collects softmax
denominators via a ones row appended to V). Wide PV matmuls (<=512 q
columns) keep the PE instruction count low — per-instruction semaphore and
issue overhead (~50-100 ns) is what kills many-small-matmul schedules. The
diagonal (causally masked) block's PV is deferred one k-chunk so the Pool
affine_select round-trip hides behind the next chunk's scores.

Schedule: the Act engine paces each head (~19 us of softmax exp vs ~16 us
of PE work), so the remaining phase-1 chains and the previous pair's
partial c_proj run as filler inside later heads' k-chunk loops. Each head's
normalization (1/den broadcast via a ones-column matmul) is deferred into
the next head's first k-chunk. The final pair's c_proj evacuates PSUM on
both DVE and Act (Act is idle by then).
"""

import numpy as np
import ml_dtypes
from contextlib import ExitStack

import concourse.bass as bass
import concourse.tile as tile
from concourse import bacc, mybir
from concourse.bass_utils import run_bass_kernel_spmd

F32 = mybir.dt.float32
F32R = mybir.dt.float32r
BF16 = mybir.dt.bfloat16
FP8 = mybir.dt.float8e4
DR = mybir.MatmulPerfMode.DoubleRow
ACT_EXP = mybir.ActivationFunctionType.Exp
ACT_COPY = mybir.ActivationFunctionType.Copy
MULT = mybir.AluOpType.mult
ADD = mybir.AluOpType.add

B, S, E, H, D, P = 4, 2048, 1024, 16, 64, 64
G = 8            # heads per core
NCORE = 8
NT = S // 128    # 16 sequence tiles
NPAIR = 5        # fp8 DoubleRow contraction pairs: 1280 = 5*256 rows (1089 used)
CSCALE = 32.0    # fp8 pre-scale on the correction matrices
INV_CS = 1.0 / CSCALE


def build_nc():
    nc = bacc.Bacc("TRN2", target_bir_lowering=False, debug=False, num_devices=NCORE)
    cqk8 = nc.dram_tensor("cqk8", [128, NPAIR, 2, 1024], FP8, kind="ExternalInput").ap()
    cv8 = nc.dram_tensor("cv8", [128, NPAIR, 2, 512], FP8, kind="ExternalInput").ap()
    xt8 = nc.dram_tensor("xt8", [128, NPAIR, 2, S], FP8, kind="ExternalInput").ap()
    dxt8 = nc.dram_tensor("dxt8", [128, 4, 2, S], FP8, kind="ExternalInput").ap()
    xtid = nc.dram_tensor("xtid", [128, 4, S], BF16, kind="ExternalInput").ap()
    xnat = nc.dram_tensor("xnat", [128, NT, 512], BF16, kind="ExternalInput").ap()
    wc16 = nc.dram_tensor("wc16", [128, 4, E], BF16, kind="ExternalInput").ap()
    onesb = nc.dram_tensor("onesb", [128, 128], BF16, kind="ExternalInput").ap()
    zerob = nc.dram_tensor("zerob", [1, 512], BF16, kind="ExternalInput").ap()
    outp = nc.dram_tensor("outp", [2, S, E], BF16, kind="ExternalOutput").ap()

    with nc.allow_low_precision("bf16/fp8 staged operands; f32 PSUM accumulation"), \
         tile.TileContext(nc) as tc, ExitStack() as top:
        const_p = top.enter_context(tc.tile_pool(name="const", bufs=1))
        qk_p = top.enter_context(tc.tile_pool(name="qkt", bufs=1))
        vaug_p = top.enter_context(tc.tile_pool(name="vaug", bufs=1))
        oT_p = top.enter_context(tc.tile_pool(name="oT", bufs=1))

        cqk_sb = const_p.tile([128, NPAIR, 2, 1024], FP8)
        cv_sb = const_p.tile([128, NPAIR, 2, 512], FP8)
        xt_sb = const_p.tile([128, NPAIR, 2, S], FP8)
        dxt_sb = const_p.tile([128, 4, 2, S], FP8)
        xtid_sb = const_p.tile([128, 4, S], BF16)
        xnat_sb = const_p.tile([128, NT, 512], BF16)
        wc_sb = const_p.tile([128, 4, E], BF16)
        zrow = const_p.tile([1, 512], BF16)

        qkt = [qk_p.tile([128, S], BF16, name=f"qkt{m}") for m in range(8)]
        v65 = vaug_p.tile([128, NT, G, 66], BF16)   # col 64 = ones (denominator)
        oT = oT_p.tile([128, 4, S], BF16)

        # --- DMAs, startup-critical first -------------------------------
        # each DMA pays ~625ns serialized HWDGE overhead + ~900ns sem
        # propagation, so transfers are consolidated (DRAM layouts mirror the
        # SBUF tiles, partition-major)
        nc.sync.dma_start(out=zrow, in_=zerob)
        nc.vector.memset(v65[:, :, :, 64:66], 1.0)
        # stage 1: what the first chains (m0/m4, all pairs) + head-0 kc0 need
        nc.sync.dma_start(out=cqk_sb[:, :, :, 0:128], in_=cqk8[:, :, :, 0:128])
        nc.sync.dma_start(out=cqk_sb[:, :, :, 512:640], in_=cqk8[:, :, :, 512:640])
        nc.sync.dma_start(out=xt_sb[:, :, :, 0:1024], in_=xt8[:, :, :, 0:1024])
        nc.sync.dma_start(out=dxt_sb[:, :, :, 0:1024], in_=dxt8[:, :, :, 0:1024])
        nc.sync.dma_start(out=xtid_sb[:, 0, 0:1024], in_=xtid[:, 0, 0:1024])
        # stage 2: second halves, v-path, remaining weights
        nc.sync.dma_start(out=xt_sb[:, :, :, 1024:2048], in_=xt8[:, :, :, 1024:2048])
        nc.sync.dma_start(out=dxt_sb[:, :, :, 1024:2048],
                          in_=dxt8[:, :, :, 1024:2048])
        nc.sync.dma_start(out=xtid_sb[:, 0, 1024:2048], in_=xtid[:, 0, 1024:2048])
        nc.sync.dma_start(out=cv_sb, in_=cv8)
        nc.sync.dma_start(out=xnat_sb[:, 0:8], in_=xnat[:, 0:8])
        nc.sync.dma_start(out=xnat_sb[:, 8:NT], in_=xnat[:, 8:NT])
        nc.sync.dma_start(out=cqk_sb[:, :, :, 128:512], in_=cqk8[:, :, :, 128:512])
        nc.sync.dma_start(out=cqk_sb[:, :, :, 640:1024],
                          in_=cqk8[:, :, :, 640:1024])
        nc.sync.dma_start(out=xtid_sb[:, 1:4], in_=xtid[:, 1:4])
        nc.sync.dma_start(out=wc_sb, in_=wc16)

        with tc.tile_pool(name="stp", bufs=2, space="PSUM") as stp_p, \
             tc.tile_pool(name="pop", bufs=1, space="PSUM") as po_p, \
             tc.tile_pool(name="pt", bufs=4) as pt_p, \
             tc.tile_pool(name="diag", bufs=2) as diag_p, \
             tc.tile_pool(name="bcst", bufs=1) as bcst_p, \
             tc.tile_pool(name="rcpp", bufs=1) as rcp_p, \
             tc.tile_pool(name="ost", bufs=3) as ost_p:

            # ---------- phase-1 building blocks ----------
            def qk_chain(m, ss):
                """qkt[m][:, ss*512:+512] = identity x-slice + fp8 correction."""
                ps = stp_p.tile([128, 1024], F32, tag="stp", name=f"qkps{m}_{ss}")
                side = 0 if m < 4 else 512
                col0 = side + (m % 4) * 128
                xsl = slice(ss * 512, (ss + 1) * 512)
                for pr in range(NPAIR):
                    nc.tensor.matmul(ps[:, 0:512],
                                     cqk_sb[:, pr, :, col0:col0 + 128],
                                     xt_sb[:, pr, :, xsl],
                                     start=(pr == 0), stop=False, perf_mode=DR)
                for pr in range(4):
                    nc.tensor.matmul(ps[:, 0:512],
                                     cqk_sb[:, pr, :, col0:col0 + 128],
                                     dxt_sb[:, pr, :, xsl],
                                     start=False, stop=(pr == 3), perf_mode=DR)
                nc.vector.scalar_tensor_tensor(
                    out=qkt[m][:, xsl], in0=ps[:, 0:512], scalar=INV_CS,
                    in1=xtid_sb[:, m % 4, xsl], op0=MULT, op1=ADD)

            def v_chain(stt):
                pv = stp_p.tile([128, 1024], F32, tag="stp", name=f"vps{stt}")
                for pr in range(NPAIR):
                    nc.tensor.matmul(pv[:, 0:512],
                                     xt_sb[:, pr, :, stt * 128:(stt + 1) * 128],
                                     cv_sb[:, pr, :, :],
                                     start=(pr == 0), stop=(pr == NPAIR - 1),
                                     perf_mode=DR)
                nc.vector.scalar_tensor_tensor(
                    out=v65[:, stt, :, 0:64],
                    in0=pv[:, 0:512].rearrange("p (g d) -> p g d", g=G),
                    scalar=INV_CS,
                    in1=xnat_sb[:, stt].rearrange("p (g d) -> p g d", g=G),
                    op0=MULT, op1=ADD)

            def cproj_chunk(part, stt, on_act=False):
                """one 128-row slab of the half c_proj (head-pairs 2p, 2p+1)"""
                pc = stp_p.tile([128, 1024], F32, tag="stp", name=f"pc{part}_{stt}")
                for gi, gc in enumerate((2 * part, 2 * part + 1)):
                    for ee in range(2):
                        nc.tensor.matmul(pc[:, ee * 512:(ee + 1) * 512],
                                         oT[:, gc, stt * 128:(stt + 1) * 128],
                                         wc_sb[:, gc, ee * 512:(ee + 1) * 512],
                                         start=(gi == 0), stop=(gi == 1),
                                         skip_group_check=True)
                ost = ost_p.tile([128, E], BF16, tag="ost", name=f"ost{part}_{stt}")
                if on_act:
                    nc.scalar.activation(ost[:, :], pc[:, :], ACT_COPY)
                else:
                    nc.vector.tensor_copy(ost[:, :], pc[:, :])
                nc.sync.dma_start(out=outp[part, stt * 128:(stt + 1) * 128, :],
                                  in_=ost[:, :])

            # ---------- phase-2 per-head attention ----------
            state = {"norm": None, "diag": None, "pv": None, "bcst": None}

            def normalize_a(h, po):
                """reciprocal of the denominator row + GPSIMD partition
                broadcast, chunked so Pool-queue neighbors (affine_select)
                are not delayed much"""
                rcp = rcp_p.tile([1, S], F32R, tag="rcp", name=f"rcp{h}")
                bcst = bcst_p.tile([64, S], F32R, tag="bcst", name=f"bcst{h}")
                nc.vector.reciprocal(rcp, po[64:65, :])
                for grp in range(4):
                    gs = slice(grp * 512, (grp + 1) * 512)
                    nc.gpsimd.partition_broadcast(bcst[:, gs], rcp[0:1, gs])
                return bcst

            def normalize_b(h, po, bcst):
                """oT[d-half, pair, :] = po[0:64, :] * (1/den) (bf16)"""
                m, half = h // 2, h % 2
                for grp in range(4):
                    gs = slice(grp * 512, (grp + 1) * 512)
                    nc.vector.tensor_mul(
                        oT[64 * half:64 * half + 64, m, gs],
                        po[0:64, gs], bcst[:, gs])

            def head_attention(h, fillers):
                """fillers: dict kc -> list of thunks emitted at that k-chunk."""
                m, half = h // 2, h % 2
                qt = qkt[m][64 * half:64 * half + 64, :]
                kt = qkt[4 + m][64 * half:64 * half + 64, :]
                po = None
                pocell = {}
                for kc in range(NT):
                    q0 = 128 * kc
                    ptile = pt_p.tile([128, 2048], BF16, tag="pt",
                                      name=f"pt{h}_{kc}")
                    for c0 in range(q0, S, 1024):
                        cw = min(1024, S - c0)
                        st = stp_p.tile([128, 1024], F32, tag="stp",
                                        name=f"st{h}_{kc}_{c0}")
                        for u0 in range(c0, c0 + cw, 512):
                            uw = min(512, c0 + cw - u0)
                            nc.tensor.matmul(st[:, u0 - c0:u0 - c0 + uw],
                                             kt[:, q0:q0 + 128], qt[:, u0:u0 + uw],
                                             start=True, stop=True)
                        nc.scalar.activation(ptile[:, c0 - q0:c0 - q0 + cw],
                                             st[:, 0:cw], ACT_EXP, scale=0.125)
                    dg = diag_p.tile([128, 128], BF16, tag="dg",
                                     name=f"dg{h}_{kc}")
                    nc.gpsimd.affine_select(
                        out=dg, in_=ptile[:, 0:128],
                        compare_op=mybir.AluOpType.is_ge, fill=0.0,
                        base=0, pattern=[[1, 128]], channel_multiplier=-1)
                    if kc == 0 and state["norm"] is not None:
                        # previous head's normalization, stage a (rcp +
                        # Pool broadcasts run under this head's first scores)
                        state["bcst"] = normalize_a(*state["norm"])
                    if kc == 1:
                        if state["norm"] is not None:
                            normalize_b(*state["norm"], state["bcst"])
                            state["norm"] = None
                        # po slot reuse is safe only after the deferred
                        # normalization above has been emitted
                        po = po_p.tile([128, S], F32, tag="po", name=f"po{h}")
                        pocell["po"] = po
                        # zero bank 0 so every PV there can accumulate with
                        # start=False (PSUM pending-zero rules)
                        nc.tensor.matmul(po[0:65, 0:512], zrow[0:1, 0:65],
                                         zrow[0:1, 0:512],
                                         start=True, stop=False,
                                         skip_group_check=True)
                    # PV runs one k-chunk behind the scores so the PE never
                    # waits on the exp it just requested: PV(kc-1) reads a
                    # ptile whose exp finished during scores(kc)
                    if state["pv"] is not None:
                        state["pv"]()
                        state["pv"] = None
                    if state["diag"] is not None:
                        state["diag"]()
                        state["diag"] = None

                    def pv_pieces(kcc=kc, ptt=ptile, hh=h):
                        poo = pocell["po"]
                        # non-diagonal PV: q in [q0+128, S), 512-aligned
                        # pieces; kc==0 pieces outside bank 0 open their
                        # accumulation group (bank 0 was pre-zeroed)
                        a = 128 * kcc + 128
                        while a < S:
                            b_ = min(S, (a // 512 + 1) * 512)
                            nc.tensor.matmul(poo[0:65, a:b_],
                                             v65[:, kcc, hh, 0:65],
                                             ptt[:, a - 128 * kcc:b_ - 128 * kcc],
                                             start=(kcc == 0 and a >= 512),
                                             stop=False, skip_group_check=True)
                            a = b_

                    state["pv"] = pv_pieces
                    state["diag"] = (
                        lambda kcc=kc, dgg=dg, hh=h: nc.tensor.matmul(
                            pocell["po"][0:65, kcc * 128:(kcc + 1) * 128],
                            v65[:, kcc, hh, 0:65], dgg[:, :],
                            start=False, stop=True, skip_group_check=True))
                    for th in fillers.get(kc, ()):
                        th()
                state["pv"]()
                state["pv"] = None
                state["diag"]()
                state["diag"] = None
                state["norm"] = (h, pocell["po"])

            # ---------- schedule ----------
            # warm the PE clock (p-state ramps over ~3us of busy time)
            # while the input DMAs land
            for w in range(12):
                wt = stp_p.tile([128, 1024], F32, tag="stp", name=f"warm{w}")
                nc.tensor.matmul(wt[:, 0:512], zrow[0:1, 0:128],
                                 zrow[0:1, 0:512], start=True, stop=True,
                                 skip_group_check=True)
            # prefix: head 0's q must be complete (scores read qt[:, q0:S]);
            # its k and v arrive narrowly (kt[:, q0:q0+128] / v65[:, kc]) so
            # later chunks stream in as filler
            for m_, ss_ in ((0, 0), (0, 1), (4, 0), (0, 2), (0, 3)):
                qk_chain(m_, ss_)
            v_chain(0)

            def QK(m, ss):
                return lambda: qk_chain(m, ss)

            def VC(*stts):
                return lambda: [v_chain(s) for s in stts]

            def CP(gc, *stts):
                return lambda: [cproj_chunk(gc, s) for s in stts]

            # filler placement rules: a chain emitted at slot kc is only
            # readable from chunk kc+1 on (reads emitted before writes see
            # stale data).  v65[:, k] is needed at PV(k); kt chunk ss at
            # scores(4*ss); q chunks must be complete before the head starts.
            fillers = {
                0: {0: [VC(1)], 1: [QK(4, 1), VC(2)], 2: [VC(3, 4)],
                    3: [VC(5, 6)], 5: [QK(4, 2), VC(7, 8)], 7: [VC(9, 10)],
                    9: [QK(4, 3), VC(11, 12)], 11: [VC(13, 14)], 13: [VC(15)]},
                1: {1: [QK(1, 0)], 3: [QK(5, 0)], 5: [QK(1, 1)], 7: [QK(5, 1)],
                    9: [QK(1, 2)], 11: [QK(5, 2)], 13: [QK(1, 3), QK(5, 3)]},
                2: {1: [QK(2, 0)], 3: [QK(6, 0)], 5: [QK(2, 1)], 7: [QK(6, 1)]},
                3: {1: [QK(2, 2)], 3: [QK(6, 2)], 5: [QK(2, 3)], 7: [QK(6, 3)]},
                4: {1: [QK(3, 0)], 3: [QK(7, 0)], 5: [QK(3, 1)], 7: [QK(7, 1)],
                    9: [CP(0, 0)], 11: [CP(0, 1)], 13: [CP(0, 2)]},
                5: {1: [QK(3, 2)], 3: [QK(7, 2)], 5: [QK(3, 3)], 7: [QK(7, 3)],
                    9: [CP(0, 3)], 11: [CP(0, 4)], 13: [CP(0, 5)]},
                6: {1: [CP(0, 6)], 3: [CP(0, 7)], 5: [CP(0, 8)], 7: [CP(0, 9)],
                    9: [CP(0, 10)], 11: [CP(0, 11)], 13: [CP(0, 12)]},
                7: {1: [CP(0, 13)], 3: [CP(0, 14)], 5: [CP(0, 15)]},
            }
            for h in range(8):
                head_attention(h, fillers[h])
            normalize_b(*state["norm"], normalize_a(*state["norm"]))
            # tail: second half of c_proj, PSUM evacuation split between DVE
            # and the now-idle Act engine
            for stt in range(NT):
                cproj_chunk(1, stt, on_act=(stt % 2 == 1))

    nc.compile()
    return nc


def _fp8(a, scale=1.0):
    return (np.asarray(a, np.float32) * scale).astype(ml_dtypes.float8_e4m3)


def _bf16(a):
    return np.asarray(a, np.float32).astype(ml_dtypes.bfloat16)


def prep_core_inputs(hidden_states, position_states, Wq, bq, Wqh, bqh, Wk, bk,
                     Wkh, bkh, Wv, bv, Wvh, bvh, Wp, bp, Wpe, bpe, Wc, bc):
    """Host-side weight folding + per-core staging."""
    f32 = np.float32
    eyeE = np.eye(E, dtype=f32)

    def fold(parity):
        hs = slice(G * parity, G * parity + G)
        csl = slice(512 * parity, 512 * parity + 512)
        mats = {}
        for name, (Wa, ba, Wh, bh, v) in {
            "q": (Wq, bq, Wqh[hs], bqh[hs], 0),
            "k": (Wk, bk, Wkh[hs], bkh[hs], 1),
            "v": (Wv, bv, Wvh[hs], bvh[hs], 2),
        }.items():
            mx = np.einsum("hed,ghd->hegd", Wa, Wh).reshape(E, 512)
            mp = np.einsum("pd,g->pgd", Wp[:, v * D:(v + 1) * D],
                           Wpe[v, 0, hs]).reshape(P, 512)
            bias = (np.einsum("hd,ghd->gd", ba, Wh) + bh
                    + bp[v * D:(v + 1) * D][None, :] * Wpe[v, 0, hs][:, None]
                    + bpe[hs][:, None]).reshape(512)
            C = np.zeros((NPAIR * 256, 512), f32)
            C[:E] = mx - eyeE[:, csl]
            C[E:E + P] = mp
            C[E + P] = bias
            mats[name] = C
        cqk = np.concatenate([mats["q"], mats["k"]], axis=1)     # [1280, 1024]
        cqk8 = _fp8(cqk.reshape(NPAIR, 2, 128, 1024).transpose(2, 0, 1, 3), CSCALE)
        cv8 = _fp8(mats["v"].reshape(NPAIR, 2, 128, 512).transpose(2, 0, 1, 3), CSCALE)
        wc = Wc.reshape(H, D, E)[hs].reshape(512, E).reshape(4, 128, E).transpose(1, 0, 2)
        return (np.ascontiguousarray(cqk8), np.ascontiguousarray(cv8),
                np.ascontiguousarray(_bf16(wc)))

    per_parity = [fold(0), fold(1)]

    in_maps = []
    for c in range(NCORE):
        b, parity = c // 2, c % 2
        csl = slice(512 * parity, 512 * parity + 512)
        xaug = np.zeros((NPAIR * 256, S), f32)
        xaug[:E] = hidden_states[b].T
        xaug[E:E + P] = position_states[b].T
        xaug[E + P] = 1.0
        x8 = _fp8(xaug)
        dx = xaug[:E] - x8[:E].astype(f32)
        xt8 = np.ascontiguousarray(
            x8.reshape(NPAIR, 2, 128, S).transpose(2, 0, 1, 3))
        dxt8 = np.ascontiguousarray(
            _fp8(dx).reshape(4, 2, 128, S).transpose(2, 0, 1, 3))
        xtid = np.ascontiguousarray(
            _bf16(hidden_states[b].T[csl]).reshape(4, 128, S).transpose(1, 0, 2))
        xnat = np.ascontiguousarray(
            _bf16(hidden_states[b][:, csl]).reshape(NT, 128, 512).transpose(1, 0, 2))
        cqk8, cv8, wc = per_parity[parity]
        in_maps.append({"cqk8": cqk8, "cv8": cv8, "xt8": xt8, "dxt8": dxt8,
                        "xtid": xtid, "xnat": xnat, "wc16": wc,
                        "onesb": _bf16(np.ones((128, 128), f32)),
                        "zerob": _bf16(np.zeros((1, 512), f32))})
    return in_maps


_NC_CACHE = {}


def get_nc():
    if "nc" not in _NC_CACHE:
        _NC_CACHE["nc"] = build_nc()
    return _NC_CACHE["nc"]


def assemble(results, bc):
    outs = []
    for b in range(B):
        acc = np.zeros((S, E), np.float32)
        for p in range(2):
            acc += results[2 * b + p]["outp"].astype(np.float32).sum(axis=0)
        outs.append(acc + bc)
    return np.stack(outs).astype(np.float32)


def kernel(**inputs):
    nc = get_nc()
    in_maps = prep_core_inputs(**inputs)
    res = run_bass_kernel_spmd(nc, in_maps, list(range(NCORE)))
    return assemble(res.results, inputs["bc"])
